# revision 2
# baseline (speedup 1.0000x reference)
"""Trainium2 Bass kernel for MCPRN (purpose-routed GRU-variant session
recommender). Two SPMD launches on 8 NeuronCores.

Launch 1 (scan): cores 0-5 run (purpose p, batch-half h) PSRU scans as two
  interleaved 32-wide chains. The hidden state is carried as two bf16
  summands (m, an) so the per-step critical path never materializes h:
    h(t) = m(t) + an(t),   m(t) = h(t-1)*(1-a(t)),   an(t) = a(t)*n(t)
  and the gate matmuls take (m, an) as two moving operands accumulating in
  PSUM. x-side gate contributions and biases are precomputed per 4-step
  group (double-buffered PSUM), sliced per step to keep the PE queue fine-
  grained. Concen weights computed on device (softmax/tau, masked, eps-
  clamped) and broadcast across partitions via a DRAM roundtrip.

Launch 2 (score): scores[b,t] = sum_p tcw[t,p] * <hn[p,b,:], emb[t,:]>
  rewritten (sum_p tcw = 1) as  P2 + w0*(P0-P2) + w1*(P1-P2):
  per item chunk, w0/w1 rows are broadcast across partitions with K=1
  ones-matmuls, copied to bf16, the emb chunk is scaled by them on DVE
  (one 4x-mode scalar_tensor_tensor), and all six K=128 matmuls (raw emb
  against hn2, scaled emb against hn_p - hn_2) accumulate into one PSUM
  tile that DMAs straight to DRAM. The softmax tcw itself (50001x3) is
  computed on host; all matmul FLOPs stay on device.
"""

import numpy as np
import ml_dtypes

import concourse.bacc as bacc
import concourse.mybir as mybir
import concourse.tile as tile
from concourse.bass import ts, ds
from concourse.bass_utils import run_bass_kernel_spmd

F32 = mybir.dt.float32
BF16 = mybir.dt.bfloat16
AF = mybir.ActivationFunctionType
OP = mybir.AluOpType

N_ITEMS = 50001
DIM = 256
TAU = 0.1
S = 50
B = 128
EPS = 0.01
W = 64          # batch per scan core
CW = 32         # chain width (two chains per core)
GS = 4          # x-side group size (steps)
SB = S * W      # 3200
NT = SB // 128  # 25
NCORES = 8

CORE_PH = [(0, 0), (0, 1), (1, 0), (1, 1), (2, 0), (2, 1), (0, 0), (0, 1)]

# scoring chunking
T_PAD = 6272            # 49 * 128, per-core padded item count
NC = 512
CHUNK_SIZES = [512] * 12 + [128]
CHUNK_OFFS = np.cumsum([0] + CHUNK_SIZES).tolist()

_BF = ml_dtypes.bfloat16

GROUPS = []
_g0 = 0
while _g0 < S:
    GROUPS.append((_g0, min(GS, S - _g0)))
    _g0 += GS


# --------------------------------------------------------------------------
# Launch 1: scan
# --------------------------------------------------------------------------

def build_scan_nc():
    nc = bacc.Bacc("TRN2", target_bir_lowering=False, debug=False,
                   num_devices=NCORES)

    wiT_d = nc.dram_tensor("wiT", [128, 2, 768], BF16, kind="ExternalInput")
    whT_d = nc.dram_tensor("whT", [128, 2, 768], BF16, kind="ExternalInput")
    xT_d = nc.dram_tensor("xT", [128, 2, SB], BF16, kind="ExternalInput")
    pT_d = nc.dram_tensor("pT", [128, 2, 3], BF16, kind="ExternalInput")
    mask_d = nc.dram_tensor("mask", [128, NT], F32, kind="ExternalInput")
    bri_d = nc.dram_tensor("bri", [1, 512], BF16, kind="ExternalInput")
    bin_d = nc.dram_tensor("bin", [1, 256], BF16, kind="ExternalInput")
    bhn_d = nc.dram_tensor("bhnr", [1, 256], BF16, kind="ExternalInput")
    hn_out = nc.dram_tensor("hn_out", [128, 2 * W], BF16, kind="ExternalOutput")
    cf_lin = nc.dram_tensor("cf_lin", [NT, 128], BF16)

    with tile.TileContext(nc) as tc:
        with (
            tc.tile_pool(name="consts", bufs=1) as consts,
            tc.tile_pool(name="cwp", bufs=1) as cwp,
            tc.tile_pool(name="gx", bufs=2, space="PSUM") as gx,
            tc.tile_pool(name="ghn", bufs=1, space="PSUM") as ghnp,
            tc.tile_pool(name="ew", bufs=3) as ew,
            tc.tile_pool(name="hpool", bufs=3) as hpool,
        ):
            pT = consts.tile_from(pT_d.ap())
            xT = consts.tile_from(xT_d.ap())
            wiT = consts.tile_from(wiT_d.ap())
            whT = consts.tile_from(whT_d.ap())
            mask = consts.tile_from(mask_d.ap())
            bri = consts.tile_from(bri_d.ap())
            bin_ = consts.tile_from(bin_d.ap())
            bhnr = consts.tile_from(bhn_d.ap())
            ones = consts.tile([1, GS * W], BF16)
            nc.vector.memset(ones[:], 1.0)

            # ---------------- concen -> cf_rep (prologue) ----------------
            # PSUM is fully budgeted for the scan; the concen matmuls borrow
            # the (much larger) x-group "gri" tag buffer before the scan
            # starts using it.
            ps_s = gx.tile([128, NT, 3], F32, tag="gri", name="ps_s")
            for tt in range(NT):
                for k in range(2):
                    nc.tensor.matmul(
                        ps_s[:, tt, :], xT[:, k, ts(tt, 128)], pT[:, k, :],
                        start=(k == 0), stop=(k == 1))
            e_s = cwp.tile([128, NT, 3], F32)
            nc.scalar.activation(e_s[:], ps_s[:], AF.Exp, scale=1.0 / TAU)
            den = cwp.tile([128, NT], F32)
            nc.vector.tensor_reduce(den[:], e_s[:], mybir.AxisListType.X,
                                    OP.add)
            rden = cwp.tile([128, NT], F32)
            nc.vector.reciprocal_approx_fast(rden[:], den[:])
            cnorm = cwp.tile([128, NT, 3], F32)
            nc.vector.tensor_tensor(
                cnorm[:], e_s[:],
                rden[:, :, None].to_broadcast((128, NT, 3)), OP.mult)
            cm = cwp.tile([128, NT, 3], F32)
            nc.vector.tensor_tensor(
                cm[:], cnorm[:],
                mask[:, :, None].to_broadcast((128, NT, 3)), OP.mult)
            ge = cwp.tile([128, NT, 3], F32)
            nc.vector.tensor_scalar(ge[:], cm[:], EPS, None, OP.is_ge)
            cf3 = cwp.tile([128, NT, 3], BF16)
            nc.vector.tensor_tensor(cf3[:], cm[:], ge[:], OP.mult)
            nc.sync.dma_start(cf_lin.ap().rearrange("t p -> p t"),
                              cf3[:, :, 0])
            cf_rep = cwp.tile([128, SB], BF16)
            nc.sync.dma_start(
                cf_rep[:],
                cf_lin.ap().rearrange("t p -> (t p)")[None, :]
                .to_broadcast((128, SB)))

            # ---------------- state ----------------
            m_t, an_t, h_t = {}, {}, {}
            for c in range(2):
                m_t[c] = hpool.tile([128, 2, CW], BF16, tag=f"m{c}",
                                    name=f"m0_{c}")
                nc.vector.memset(m_t[c][:], 0.0)
                h_t[c] = hpool.tile([128, 2, CW], BF16, tag=f"h{c}",
                                    name=f"h0_{c}")
                nc.vector.memset(h_t[c][:], 0.0)
                an_t[c] = hpool.tile([128, 2, CW], BF16, tag=f"an{c}",
                                     name=f"an0_{c}")
                nc.vector.memset(an_t[c][:], 0.0)

            def xstripe(gri, gin, g0, gn, stripes):
                gw = gn * W
                col = ds(g0 * W, gw)
                for s in stripes:
                    if s < 4:
                        j = s
                        nc.tensor.matmul(gri[:, j, :gn, :],
                                         bri[0:1, ts(j, 128)],
                                         ones[0:1, :gw], start=True,
                                         stop=False)
                        for k in range(2):
                            nc.tensor.matmul(
                                gri[:, j, :gn, :], wiT[:, k, ts(j, 128)],
                                xT[:, k, col], start=False, stop=(k == 1))
                    else:
                        j = s - 4
                        nc.tensor.matmul(gin[:, j, :gn, :],
                                         bin_[0:1, ts(j, 128)],
                                         ones[0:1, :gw], start=True,
                                         stop=False)
                        for k in range(2):
                            nc.tensor.matmul(
                                gin[:, j, :gn, :],
                                wiT[:, k, ds(512 + j * 128, 128)],
                                xT[:, k, col], start=False, stop=(k == 1))

            def newgroup():
                gri = gx.tile([128, 4, GS, W], F32, tag="gri", name="gri")
                gin = gx.tile([128, 2, GS, W], F32, tag="gin", name="gin")
                return gri, gin

            def gin_copy(gin, gn):
                gs_sb = ew.tile([128, 2, GS, W], BF16, tag="ginS", name="ginS")
                nc.vector.tensor_copy(gs_sb[:, :, :gn, :], gin[:, :, :gn, :])
                return gs_sb

            cur = newgroup()
            xstripe(*cur, GROUPS[0][0], GROUPS[0][1], range(6))
            cur_sb = gin_copy(cur[1], GROUPS[0][1])
            nxt = newgroup() if len(GROUPS) > 1 else None
            # stripes of the next group, spread over this group's steps
            STRIPE_SCHED = {0: (0, 1), 1: (2, 3), 2: (4,), 3: (5,)}

            for gi, (g0, gn) in enumerate(GROUPS):
                gri, gin = cur
                for tl in range(gn):
                    t = g0 + tl
                    for c in range(2):
                        bsl = ds(c * CW, CW)
                        ps_ghn = ghnp.tile([128, 2, CW], F32, tag=f"ghn{c}",
                                           name=f"ghn{c}")
                        # r tiles: m-src first (ready early), then an-src
                        for src in (m_t[c], an_t[c]):
                            for j in range(2):
                                for k in range(2):
                                    nc.tensor.matmul(
                                        gri[:, j, tl, bsl],
                                        whT[:, k, ts(j, 128)],
                                        src[:, k, :], start=False, stop=False,
                                        skip_group_check=True)
                        for src in (m_t[c], an_t[c]):
                            for j in range(2, 4):
                                for k in range(2):
                                    nc.tensor.matmul(
                                        gri[:, j, tl, bsl],
                                        whT[:, k, ts(j, 128)],
                                        src[:, k, :], start=False, stop=False,
                                        skip_group_check=True)
                        for j in range(2):
                            nc.tensor.matmul(
                                ps_ghn[:, j, :], bhnr[0:1, ts(j, 128)],
                                ones[0:1, :CW], start=True, stop=False)
                            for si, src in enumerate((m_t[c], an_t[c])):
                                for k in range(2):
                                    nc.tensor.matmul(
                                        ps_ghn[:, j, :],
                                        whT[:, k, ds(512 + j * 128, 128)],
                                        src[:, k, :],
                                        start=False,
                                        stop=(si == 1 and k == 1))
                        sri = ew.tile([128, 4, CW], BF16, tag=f"sri{c}",
                                      name=f"sri{c}")
                        nc.scalar.activation(sri[:], gri[:, :, tl, bsl],
                                             AF.Sigmoid)
                        u1 = ew.tile([128, 2, CW], BF16, tag=f"u1{c}",
                                     name=f"u1{c}")
                        nc.vector.tensor_tensor(
                            u1[:], ps_ghn[:], sri[:, 0:2, :], OP.mult)
                        u2 = ew.tile([128, 2, CW], BF16, tag=f"u2{c}",
                                     name=f"u2{c}")
                        nc.vector.tensor_tensor(u2[:], u1[:],
                                                cur_sb[:, :, tl, bsl], OP.add)
                        a_t = ew.tile([128, 2, CW], BF16, tag=f"a{c}",
                                      name=f"a{c}")
                        nc.vector.tensor_tensor(
                            a_t[:], sri[:, 2:4, :],
                            cf_rep[:, None, ds(t * W + c * CW, CW)]
                            .to_broadcast((128, 2, CW)), OP.mult)
                        n_t = ew.tile([128, 2, CW], BF16, tag=f"n{c}",
                                      name=f"n{c}")
                        nc.scalar.activation(n_t[:], u2[:], AF.Tanh)
                        # off-critical: q, m (Pool)
                        q_t = ew.tile([128, 2, CW], BF16, tag=f"q{c}",
                                      name=f"q{c}")
                        nc.gpsimd.tensor_scalar(q_t[:], a_t[:], -1.0, 1.0,
                                                OP.mult, OP.add)
                        m_new = hpool.tile([128, 2, CW], BF16, tag=f"m{c}",
                                           name=f"m{c}")
                        nc.gpsimd.tensor_tensor(m_new[:], h_t[c][:], q_t[:],
                                                OP.mult)
                        # critical: an
                        an_new = hpool.tile([128, 2, CW], BF16, tag=f"an{c}",
                                            name=f"an{c}")
                        nc.vector.tensor_tensor(an_new[:], a_t[:], n_t[:],
                                                OP.mult)
                        # off-critical: h materialization (Pool)
                        h_new = hpool.tile([128, 2, CW], BF16, tag=f"h{c}",
                                           name=f"h{c}")
                        nc.gpsimd.tensor_tensor(h_new[:], m_new[:], an_new[:],
                                                OP.add)
                        m_t[c], an_t[c], h_t[c] = m_new, an_new, h_new
                    # prefetch stripes of the next x group
                    if nxt is not None and gi + 1 < len(GROUPS):
                        g0n, gnn = GROUPS[gi + 1]
                        sched = STRIPE_SCHED.get(tl, ())
                        if gn < GS and tl == gn - 1:
                            sched = tuple(s for tl2 in range(tl, GS)
                                          for s in STRIPE_SCHED.get(tl2, ()))
                        xstripe(*nxt, g0n, gnn, sched)
                if gi + 1 < len(GROUPS):
                    cur = nxt
                    cur_sb = gin_copy(cur[1], GROUPS[gi + 1][1])
                    nxt = newgroup() if gi + 2 < len(GROUPS) else None

            for c in range(2):
                nc.sync.dma_start(
                    hn_out.ap().rearrange("p (k b) -> p k b", k=2)
                    [:, :, ds(c * CW, CW)], h_t[c][:])

    nc.compile()
    return nc


def scan_host_inputs(seq, emb, emb_purpose, w_ih, w_hh, b_ih, b_hh):
    seq = np.asarray(seq)
    xg = emb[seq]                      # [S, B, D] gather (input staging)
    in_maps = []
    for c in range(NCORES):
        p, h = CORE_PH[c]
        sl = slice(h * W, (h + 1) * W)
        xh = xg[:, sl, :]              # [S, W, D]
        xT = np.ascontiguousarray(
            xh.transpose(2, 0, 1).reshape(2, 128, SB).transpose(1, 0, 2))
        wiT = np.ascontiguousarray(
            w_ih[p].T.reshape(2, 128, 768).transpose(1, 0, 2))
        whT = np.ascontiguousarray(
            w_hh[p].T.reshape(2, 128, 768).transpose(1, 0, 2))
        perm = [p, (p + 1) % 3, (p + 2) % 3]
        pT = np.ascontiguousarray(
            emb_purpose[perm].T.reshape(2, 128, 3).transpose(1, 0, 2))
        m = (seq[:, sl] != 0).astype(np.float32).reshape(SB)
        mask = np.ascontiguousarray(m.reshape(NT, 128).T)
        bsum = (b_ih[p] + b_hh[p])[:512]
        in_maps.append({
            "wiT": wiT.astype(_BF), "whT": whT.astype(_BF),
            "xT": xT.astype(_BF),
            "pT": pT.astype(_BF), "mask": mask,
            "bri": bsum[None, :].astype(_BF),
            "bin": b_ih[p][None, 512:].astype(_BF),
            "bhnr": b_hh[p][None, 512:].astype(_BF),
        })
    return in_maps


# --------------------------------------------------------------------------
# Launch 2: scoring
# --------------------------------------------------------------------------

def build_score_nc():
    nc = bacc.Bacc("TRN2", target_bir_lowering=False, debug=False,
                   num_devices=NCORES)

    # hT6: [128, 6, 128] = (d-part, [hn2 k0, hn2 k1, d0 k0, d0 k1, d1 k0,
    #                                d1 k1], b) where d_p = hn_p - hn_2
    hT6_d = nc.dram_tensor("hT6", [128, 6, 128], BF16, kind="ExternalInput")
    eT_d = nc.dram_tensor("eT", [128, 2, T_PAD], BF16, kind="ExternalInput")
    w01_d = nc.dram_tensor("w01", [1, 2, T_PAD], BF16, kind="ExternalInput")
    scores_d = nc.dram_tensor("scores", [128, T_PAD], F32,
                              kind="ExternalOutput")

    with tile.TileContext(nc) as tc:
        with (
            tc.tile_pool(name="consts", bufs=1) as consts,
            tc.tile_pool(name="wpsum", bufs=2, space="PSUM") as wpsum,
            tc.tile_pool(name="spsum", bufs=2, space="PSUM") as spsum,
            tc.tile_pool(name="work", bufs=3) as work,
        ):
            hT6 = consts.tile_from(hT6_d.ap())
            eT = consts.tile_from(eT_d.ap())
            w01 = consts.tile_from(w01_d.ap())
            onek = consts.tile([1, 128], BF16)
            nc.vector.memset(onek[:], 1.0)

            for ci, (c0, cs) in enumerate(zip(CHUNK_OFFS[:-1], CHUNK_SIZES)):
                # broadcast w0,w1 rows across partitions (K=1 ones-matmul)
                ps_w = wpsum.tile([128, 2, NC], F32, tag="w", name="ps_w")
                for p in range(2):
                    nc.tensor.matmul(ps_w[:, p, :cs], onek[0:1, :],
                                     w01[0:1, p, ds(c0, cs)],
                                     start=True, stop=True)
                w_sb = work.tile([128, 2, NC], BF16, tag="wsb", name="w_sb")
                nc.scalar.activation(w_sb[:, :, :cs], ps_w[:, :, :cs], AF.Copy)
                # scaled tables: esc[p,k,:] = eT[k,:] * w_p
                # (4 plain TTs, no broadcast APs, so DVE 2x mode applies;
                #  one TT on Pool to balance engines)
                esc = work.tile([128, 2, 2, NC], BF16, tag="esc", name="esc")
                for p in range(2):
                    for k in range(2):
                        eng = nc.gpsimd if (p == 1 and k == 1) else nc.vector
                        eng.tensor_tensor(
                            esc[:, p, k, :cs], eT[:, k, ds(c0, cs)],
                            w_sb[:, p, :cs], OP.mult)
                # scores = P2 + w0*dP0 + w1*dP1, all in one PSUM accumulation
                ps_s = spsum.tile([128, NC], F32, tag="s", name="ps_s")
                for k in range(2):
                    nc.tensor.matmul(ps_s[:, :cs], hT6[:, k, :],
                                     eT[:, k, ds(c0, cs)],
                                     start=(k == 0), stop=False)
                for p in range(2):
                    for k in range(2):
                        nc.tensor.matmul(
                            ps_s[:, :cs], hT6[:, 2 + p * 2 + k, :],
                            esc[:, p, k, :cs],
                            start=False, stop=(p == 1 and k == 1))
                out_c = work.tile([128, NC], F32, tag="out", name="out_c")
                nc.vector.tensor_copy(out_c[:, :cs], ps_s[:, :cs])
                nc.sync.dma_start(scores_d.ap()[:, ds(c0, cs)], out_c[:, :cs])

    nc.compile()
    return nc


def score_host_inputs(hn_bf, emb, emb_purpose):
    # host: tcw softmax (z = emb @ ep.T is 0.2% of total FLOPs) + hn deltas
    z = emb @ emb_purpose.T                      # [T, 3] f32
    z = z - z.max(axis=1, keepdims=True)
    ez = np.exp(z)
    w = ez / ez.sum(axis=1, keepdims=True)       # tcw

    hn = hn_bf.astype(np.float32)                # [3, B, D]
    h2 = hn[2]
    d0 = hn[0] - h2
    d1 = hn[1] - h2
    # hT6 [128, 6, 128]: stationary tiles [d-part, slot, b]
    hT6 = np.zeros((128, 6, 128), _BF)
    for k in range(2):
        hT6[:, 0 + k, :] = h2.T[k * 128:(k + 1) * 128, :].astype(_BF)
        hT6[:, 2 + k, :] = d0.T[k * 128:(k + 1) * 128, :].astype(_BF)
        hT6[:, 4 + k, :] = d1.T[k * 128:(k + 1) * 128, :].astype(_BF)

    embT = emb.T.astype(_BF)  # [256, 50001]

    base = N_ITEMS // NCORES
    rem = N_ITEMS - base * NCORES
    bounds = []
    s0 = 0
    for c in range(NCORES):
        n = base + (1 if c < rem else 0)
        bounds.append((s0, s0 + n))
        s0 += n

    in_maps = []
    for c in range(NCORES):
        lo, hi = bounds[c]
        n = hi - lo
        eT = np.zeros((128, 2, T_PAD), _BF)
        eT[:, :, :n] = embT[:, lo:hi].reshape(2, 128, n).transpose(1, 0, 2)
        w01 = np.zeros((1, 2, T_PAD), _BF)
        w01[0, :, :n] = w[lo:hi, 0:2].T.astype(_BF)
        in_maps.append({"hT6": hT6, "eT": eT, "w01": w01})
    return in_maps, bounds


# --------------------------------------------------------------------------
# Entry point
# --------------------------------------------------------------------------

_SCAN_NC = None
_SCORE_NC = None


def _get_ncs():
    global _SCAN_NC, _SCORE_NC
    if _SCAN_NC is None:
        _SCAN_NC = build_scan_nc()
    if _SCORE_NC is None:
        _SCORE_NC = build_score_nc()
    return _SCAN_NC, _SCORE_NC


def kernel(seq, emb, emb_purpose, w_ih, w_hh, b_ih, b_hh):
    seq = np.asarray(seq)
    emb = np.asarray(emb, np.float32)
    emb_purpose = np.asarray(emb_purpose, np.float32)
    w_ih = np.asarray(w_ih, np.float32)
    w_hh = np.asarray(w_hh, np.float32)
    b_ih = np.asarray(b_ih, np.float32)
    b_hh = np.asarray(b_hh, np.float32)

    scan_nc, score_nc = _get_ncs()

    scan_ins = scan_host_inputs(seq, emb, emb_purpose, w_ih, w_hh, b_ih, b_hh)
    res1 = run_bass_kernel_spmd(scan_nc, scan_ins, core_ids=list(range(NCORES)))

    hn = np.zeros((3, B, DIM), _BF)
    for c in range(6):
        p, h = CORE_PH[c]
        sl = res1.results[c]["hn_out"].reshape(128, 2, W)
        for k in range(2):
            hn[p, h * W:(h + 1) * W, k * 128:(k + 1) * 128] = sl[:, k, :].T

    score_ins, bounds = score_host_inputs(hn, emb, emb_purpose)
    res2 = run_bass_kernel_spmd(score_nc, score_ins,
                                core_ids=list(range(NCORES)))

    scores = np.empty((B, N_ITEMS), np.float32)
    for c in range(NCORES):
        lo, hi = bounds[c]
        scores[:, lo:hi] = res2.results[c]["scores"][:, : hi - lo]
    return scores


# revision 3
# speedup vs baseline: 1.0708x; 1.0708x over previous
"""Trainium2 Bass kernel for MCPRN (purpose-routed GRU-variant session
recommender). Two SPMD launches on 8 NeuronCores.

Launch 1 (scan): cores 0-5 run (purpose p, batch-half h) PSRU scans as two
  interleaved 32-wide chains. The hidden state is carried as two bf16
  summands (m, an) so the per-step critical path never materializes h:
    h(t) = m(t) + an(t),   m(t) = h(t-1)*(1-a(t)),   an(t) = a(t)*n(t)
  and the gate matmuls take (m, an) as two moving operands accumulating in
  PSUM. x-side gate contributions and biases are precomputed per 4-step
  group (double-buffered PSUM), sliced per step to keep the PE queue fine-
  grained. Concen weights computed on device (softmax/tau, masked, eps-
  clamped) and broadcast across partitions via a DRAM roundtrip.

Launch 2 (score): scores[b,t] = sum_p tcw[t,p] * <hn[p,b,:], emb[t,:]>
  rewritten (sum_p tcw = 1) as  P2 + w0*(P0-P2) + w1*(P1-P2):
  per item chunk, w0/w1 rows are broadcast across partitions with K=1
  ones-matmuls, copied to bf16, the emb chunk is scaled by them on DVE
  (one 4x-mode scalar_tensor_tensor), and all six K=128 matmuls (raw emb
  against hn2, scaled emb against hn_p - hn_2) accumulate into one PSUM
  tile that DMAs straight to DRAM. The softmax tcw itself (50001x3) is
  computed on host; all matmul FLOPs stay on device.
"""

import numpy as np
import ml_dtypes

import concourse.bacc as bacc
import concourse.mybir as mybir
import concourse.tile as tile
from concourse.bass import ts, ds
from concourse.bass_utils import run_bass_kernel_spmd

F32 = mybir.dt.float32
BF16 = mybir.dt.bfloat16
AF = mybir.ActivationFunctionType
OP = mybir.AluOpType

N_ITEMS = 50001
DIM = 256
TAU = 0.1
S = 50
B = 128
EPS = 0.01
W = 64          # batch per scan core
CW = 32         # chain width (two chains per core)
GS = 4          # x-side group size (steps)
SB = S * W      # 3200
NT = SB // 128  # 25
NCORES = 8

CORE_PH = [(0, 0), (0, 1), (1, 0), (1, 1), (2, 0), (2, 1), (0, 0), (0, 1)]

# scoring chunking
T_PAD = 6272            # 49 * 128, per-core padded item count
NC = 512
CHUNK_SIZES = [512] * 12 + [128]
CHUNK_OFFS = np.cumsum([0] + CHUNK_SIZES).tolist()

_BF = ml_dtypes.bfloat16

GROUPS = []
_g0 = 0
while _g0 < S:
    GROUPS.append((_g0, min(GS, S - _g0)))
    _g0 += GS


# --------------------------------------------------------------------------
# Launch 1: scan
# --------------------------------------------------------------------------

def build_scan_nc():
    nc = bacc.Bacc("TRN2", target_bir_lowering=False, debug=False,
                   num_devices=NCORES)

    wiT_d = nc.dram_tensor("wiT", [128, 2, 768], BF16, kind="ExternalInput")
    whT_d = nc.dram_tensor("whT", [128, 2, 768], BF16, kind="ExternalInput")
    xT_d = nc.dram_tensor("xT", [128, 2, SB], BF16, kind="ExternalInput")
    pT_d = nc.dram_tensor("pT", [128, 2, 3], BF16, kind="ExternalInput")
    mask_d = nc.dram_tensor("mask", [128, NT], F32, kind="ExternalInput")
    bri_d = nc.dram_tensor("bri", [1, 512], BF16, kind="ExternalInput")
    bin_d = nc.dram_tensor("bin", [1, 256], BF16, kind="ExternalInput")
    bhn_d = nc.dram_tensor("bhnr", [1, 256], BF16, kind="ExternalInput")
    hn_out = nc.dram_tensor("hn_out", [128, 2 * W], BF16, kind="ExternalOutput")
    cf_lin = nc.dram_tensor("cf_lin", [NT, 128], BF16)

    with tile.TileContext(nc) as tc:
        with (
            tc.tile_pool(name="consts", bufs=1) as consts,
            tc.tile_pool(name="cwp", bufs=1) as cwp,
            tc.tile_pool(name="gx", bufs=2, space="PSUM") as gx,
            tc.tile_pool(name="ghn", bufs=1, space="PSUM") as ghnp,
            tc.tile_pool(name="ew", bufs=3) as ew,
            tc.tile_pool(name="hpool", bufs=3) as hpool,
        ):
            pT = consts.tile_from(pT_d.ap())
            xT = consts.tile_from(xT_d.ap())
            wiT = consts.tile_from(wiT_d.ap())
            whT = consts.tile_from(whT_d.ap())
            mask = consts.tile_from(mask_d.ap())
            bri = consts.tile_from(bri_d.ap())
            bin_ = consts.tile_from(bin_d.ap())
            bhnr = consts.tile_from(bhn_d.ap())
            ones = consts.tile([1, GS * W], BF16)
            nc.vector.memset(ones[:], 1.0)

            # ---------------- concen -> cf_rep (prologue) ----------------
            # PSUM is fully budgeted for the scan; the concen matmuls borrow
            # the (much larger) x-group "gri" tag buffer before the scan
            # starts using it.
            ps_s = gx.tile([128, NT, 3], F32, tag="gri", name="ps_s")
            for tt in range(NT):
                for k in range(2):
                    nc.tensor.matmul(
                        ps_s[:, tt, :], xT[:, k, ts(tt, 128)], pT[:, k, :],
                        start=(k == 0), stop=(k == 1))
            e_s = cwp.tile([128, NT, 3], F32)
            nc.scalar.activation(e_s[:], ps_s[:], AF.Exp, scale=1.0 / TAU)
            den = cwp.tile([128, NT], F32)
            nc.vector.tensor_reduce(den[:], e_s[:], mybir.AxisListType.X,
                                    OP.add)
            rden = cwp.tile([128, NT], F32)
            nc.vector.reciprocal_approx_fast(rden[:], den[:])
            cnorm = cwp.tile([128, NT, 3], F32)
            nc.vector.tensor_tensor(
                cnorm[:], e_s[:],
                rden[:, :, None].to_broadcast((128, NT, 3)), OP.mult)
            cm = cwp.tile([128, NT, 3], F32)
            nc.vector.tensor_tensor(
                cm[:], cnorm[:],
                mask[:, :, None].to_broadcast((128, NT, 3)), OP.mult)
            ge = cwp.tile([128, NT, 3], F32)
            nc.vector.tensor_scalar(ge[:], cm[:], EPS, None, OP.is_ge)
            cf3 = cwp.tile([128, NT, 3], BF16)
            nc.vector.tensor_tensor(cf3[:], cm[:], ge[:], OP.mult)
            nc.sync.dma_start(cf_lin.ap().rearrange("t p -> p t"),
                              cf3[:, :, 0])
            cf_rep = cwp.tile([128, SB], BF16)
            nc.sync.dma_start(
                cf_rep[:],
                cf_lin.ap().rearrange("t p -> (t p)")[None, :]
                .to_broadcast((128, SB)))

            # ---------------- state ----------------
            m_t, an_t, h_t = {}, {}, {}
            for c in range(2):
                m_t[c] = hpool.tile([128, 2, CW], BF16, tag=f"m{c}",
                                    name=f"m0_{c}")
                nc.vector.memset(m_t[c][:], 0.0)
                h_t[c] = hpool.tile([128, 2, CW], BF16, tag=f"h{c}",
                                    name=f"h0_{c}")
                nc.vector.memset(h_t[c][:], 0.0)
                an_t[c] = hpool.tile([128, 2, CW], BF16, tag=f"an{c}",
                                     name=f"an0_{c}")
                nc.vector.memset(an_t[c][:], 0.0)

            def xstripe(gri, gin, g0, gn, stripes):
                gw = gn * W
                col = ds(g0 * W, gw)
                for s in stripes:
                    if s < 4:
                        j = s
                        nc.tensor.matmul(gri[:, j, :gn, :],
                                         bri[0:1, ts(j, 128)],
                                         ones[0:1, :gw], start=True,
                                         stop=False)
                        for k in range(2):
                            nc.tensor.matmul(
                                gri[:, j, :gn, :], wiT[:, k, ts(j, 128)],
                                xT[:, k, col], start=False, stop=(k == 1))
                    else:
                        j = s - 4
                        nc.tensor.matmul(gin[:, j, :gn, :],
                                         bin_[0:1, ts(j, 128)],
                                         ones[0:1, :gw], start=True,
                                         stop=False)
                        for k in range(2):
                            nc.tensor.matmul(
                                gin[:, j, :gn, :],
                                wiT[:, k, ds(512 + j * 128, 128)],
                                xT[:, k, col], start=False, stop=(k == 1))

            def newgroup():
                gri = gx.tile([128, 4, GS, W], F32, tag="gri", name="gri")
                gin = gx.tile([128, 2, GS, W], F32, tag="gin", name="gin")
                return gri, gin

            def gin_copy(gin, gn):
                gs_sb = ew.tile([128, 2, GS, W], BF16, tag="ginS", name="ginS")
                nc.vector.tensor_copy(gs_sb[:, :, :gn, :], gin[:, :, :gn, :])
                return gs_sb

            cur = newgroup()
            xstripe(*cur, GROUPS[0][0], GROUPS[0][1], range(6))
            cur_sb = gin_copy(cur[1], GROUPS[0][1])
            nxt = newgroup() if len(GROUPS) > 1 else None
            # stripes of the next group, spread over this group's steps
            STRIPE_SCHED = {0: (0, 1), 1: (2, 3), 2: (4,), 3: (5,)}

            for gi, (g0, gn) in enumerate(GROUPS):
                gri, gin = cur
                for tl in range(gn):
                    t = g0 + tl
                    for c in range(2):
                        bsl = ds(c * CW, CW)
                        ps_ghn = ghnp.tile([128, 2, CW], F32, tag=f"ghn{c}",
                                           name=f"ghn{c}")
                        # r tiles: m-src first (ready early), then an-src
                        for src in (m_t[c], an_t[c]):
                            for j in range(2):
                                for k in range(2):
                                    nc.tensor.matmul(
                                        gri[:, j, tl, bsl],
                                        whT[:, k, ts(j, 128)],
                                        src[:, k, :], start=False, stop=False,
                                        skip_group_check=True)
                        for src in (m_t[c], an_t[c]):
                            for j in range(2, 4):
                                for k in range(2):
                                    nc.tensor.matmul(
                                        gri[:, j, tl, bsl],
                                        whT[:, k, ts(j, 128)],
                                        src[:, k, :], start=False, stop=False,
                                        skip_group_check=True)
                        for j in range(2):
                            nc.tensor.matmul(
                                ps_ghn[:, j, :], bhnr[0:1, ts(j, 128)],
                                ones[0:1, :CW], start=True, stop=False)
                            for si, src in enumerate((m_t[c], an_t[c])):
                                for k in range(2):
                                    nc.tensor.matmul(
                                        ps_ghn[:, j, :],
                                        whT[:, k, ds(512 + j * 128, 128)],
                                        src[:, k, :],
                                        start=False,
                                        stop=(si == 1 and k == 1))
                        sri = ew.tile([128, 4, CW], BF16, tag=f"sri{c}",
                                      name=f"sri{c}")
                        nc.scalar.activation(sri[:], gri[:, :, tl, bsl],
                                             AF.Sigmoid)
                        u1 = ew.tile([128, 2, CW], BF16, tag=f"u1{c}",
                                     name=f"u1{c}")
                        nc.vector.tensor_tensor(
                            u1[:], ps_ghn[:], sri[:, 0:2, :], OP.mult)
                        u2 = ew.tile([128, 2, CW], BF16, tag=f"u2{c}",
                                     name=f"u2{c}")
                        nc.vector.tensor_tensor(u2[:], u1[:],
                                                cur_sb[:, :, tl, bsl], OP.add)
                        a_t = ew.tile([128, 2, CW], BF16, tag=f"a{c}",
                                      name=f"a{c}")
                        nc.vector.tensor_tensor(
                            a_t[:], sri[:, 2:4, :],
                            cf_rep[:, None, ds(t * W + c * CW, CW)]
                            .to_broadcast((128, 2, CW)), OP.mult)
                        n_t = ew.tile([128, 2, CW], BF16, tag=f"n{c}",
                                      name=f"n{c}")
                        nc.scalar.activation(n_t[:], u2[:], AF.Tanh)
                        # off-critical: q, m (Pool)
                        q_t = ew.tile([128, 2, CW], BF16, tag=f"q{c}",
                                      name=f"q{c}")
                        nc.gpsimd.tensor_scalar(q_t[:], a_t[:], -1.0, 1.0,
                                                OP.mult, OP.add)
                        m_new = hpool.tile([128, 2, CW], BF16, tag=f"m{c}",
                                           name=f"m{c}")
                        nc.gpsimd.tensor_tensor(m_new[:], h_t[c][:], q_t[:],
                                                OP.mult)
                        # critical: an
                        an_new = hpool.tile([128, 2, CW], BF16, tag=f"an{c}",
                                            name=f"an{c}")
                        nc.vector.tensor_tensor(an_new[:], a_t[:], n_t[:],
                                                OP.mult)
                        # off-critical: h materialization (Pool)
                        h_new = hpool.tile([128, 2, CW], BF16, tag=f"h{c}",
                                           name=f"h{c}")
                        nc.gpsimd.tensor_tensor(h_new[:], m_new[:], an_new[:],
                                                OP.add)
                        m_t[c], an_t[c], h_t[c] = m_new, an_new, h_new
                    # prefetch stripes of the next x group
                    if nxt is not None and gi + 1 < len(GROUPS):
                        g0n, gnn = GROUPS[gi + 1]
                        sched = STRIPE_SCHED.get(tl, ())
                        if gn < GS and tl == gn - 1:
                            sched = tuple(s for tl2 in range(tl, GS)
                                          for s in STRIPE_SCHED.get(tl2, ()))
                        xstripe(*nxt, g0n, gnn, sched)
                if gi + 1 < len(GROUPS):
                    cur = nxt
                    cur_sb = gin_copy(cur[1], GROUPS[gi + 1][1])
                    nxt = newgroup() if gi + 2 < len(GROUPS) else None

            for c in range(2):
                nc.sync.dma_start(
                    hn_out.ap().rearrange("p (k b) -> p k b", k=2)
                    [:, :, ds(c * CW, CW)], h_t[c][:])

    nc.compile()
    return nc


def scan_host_inputs(seq, emb, emb_purpose, w_ih, w_hh, b_ih, b_hh):
    seq = np.asarray(seq)
    xg = emb[seq]                      # [S, B, D] gather (input staging)
    in_maps = []
    for c in range(NCORES):
        p, h = CORE_PH[c]
        sl = slice(h * W, (h + 1) * W)
        xh = xg[:, sl, :]              # [S, W, D]
        xT = np.ascontiguousarray(
            xh.transpose(2, 0, 1).reshape(2, 128, SB).transpose(1, 0, 2))
        wiT = np.ascontiguousarray(
            w_ih[p].T.reshape(2, 128, 768).transpose(1, 0, 2))
        whT = np.ascontiguousarray(
            w_hh[p].T.reshape(2, 128, 768).transpose(1, 0, 2))
        perm = [p, (p + 1) % 3, (p + 2) % 3]
        pT = np.ascontiguousarray(
            emb_purpose[perm].T.reshape(2, 128, 3).transpose(1, 0, 2))
        m = (seq[:, sl] != 0).astype(np.float32).reshape(SB)
        mask = np.ascontiguousarray(m.reshape(NT, 128).T)
        bsum = (b_ih[p] + b_hh[p])[:512]
        in_maps.append({
            "wiT": wiT.astype(_BF), "whT": whT.astype(_BF),
            "xT": xT.astype(_BF),
            "pT": pT.astype(_BF), "mask": mask,
            "bri": bsum[None, :].astype(_BF),
            "bin": b_ih[p][None, 512:].astype(_BF),
            "bhnr": b_hh[p][None, 512:].astype(_BF),
        })
    return in_maps


# --------------------------------------------------------------------------
# Launch 2: scoring
# --------------------------------------------------------------------------

def build_score_nc():
    nc = bacc.Bacc("TRN2", target_bir_lowering=False, debug=False,
                   num_devices=NCORES)

    # hT6: [128, 6, 128] = (d-part, [hn2 k0, hn2 k1, d0 k0, d0 k1, d1 k0,
    #                                d1 k1], b) where d_p = hn_p - hn_2
    hT6_d = nc.dram_tensor("hT6", [128, 6, 128], BF16, kind="ExternalInput")
    eT_d = nc.dram_tensor("eT", [128, 2, T_PAD], BF16, kind="ExternalInput")
    w01_d = nc.dram_tensor("w01", [1, 2, T_PAD], BF16, kind="ExternalInput")
    scores_d = nc.dram_tensor("scores", [128, T_PAD], F32,
                              kind="ExternalOutput")

    QBOUNDS = [0, 1536, 3072, 4608, T_PAD]

    with tile.TileContext(nc) as tc:
        with (
            tc.tile_pool(name="consts", bufs=1) as consts,
            tc.tile_pool(name="wpsum", bufs=2, space="PSUM") as wpsum,
            tc.tile_pool(name="spsum", bufs=2, space="PSUM") as spsum,
            tc.tile_pool(name="work", bufs=4) as work,
        ):
            hT6 = consts.tile_from(hT6_d.ap())
            w01 = consts.tile_from(w01_d.ap())
            onek = consts.tile([1, 128], BF16)
            nc.vector.memset(onek[:], 1.0)
            # quarter the eT load so chunk 0 can start after ~1/4 of the DMA
            eT_q = []
            for q in range(4):
                lo, hi = QBOUNDS[q], QBOUNDS[q + 1]
                eq = consts.tile([128, 2, hi - lo], BF16, name=f"eq{q}")
                nc.sync.dma_start(eq[:], eT_d.ap()[:, :, ds(lo, hi - lo)])
                eT_q.append(eq)

            def esl(c0, cs):
                q = 0
                while QBOUNDS[q + 1] <= c0:
                    q += 1
                assert c0 + cs <= QBOUNDS[q + 1]
                return eT_q[q][:, :, ds(c0 - QBOUNDS[q], cs)]

            for ci, (c0, cs) in enumerate(zip(CHUNK_OFFS[:-1], CHUNK_SIZES)):
                # broadcast w0,w1 rows across partitions (K=1 ones-matmul)
                ps_w = wpsum.tile([128, 2, NC], F32, tag="w", name="ps_w")
                for p in range(2):
                    nc.tensor.matmul(ps_w[:, p, :cs], onek[0:1, :],
                                     w01[0:1, p, ds(c0, cs)],
                                     start=True, stop=True)
                w_sb = work.tile([128, 2, NC], BF16, tag="wsb", name="w_sb")
                nc.scalar.activation(w_sb[:, :, :cs], ps_w[:, :, :cs], AF.Copy)
                # scaled tables: esc[p,k,:] = eT[k,:] * w_p
                # (4 plain TTs, no broadcast APs, so DVE 2x mode applies;
                #  one TT on Pool to balance engines)
                esc = work.tile([128, 2, 2, NC], BF16, tag="esc", name="esc")
                echunk = esl(c0, cs)
                for p in range(2):
                    for k in range(2):
                        eng = nc.gpsimd if (p == 1 and k == 1) else nc.vector
                        eng.tensor_tensor(
                            esc[:, p, k, :cs], echunk[:, k, :],
                            w_sb[:, p, :cs], OP.mult)
                # scores = P2 + w0*dP0 + w1*dP1, all in one PSUM accumulation
                ps_s = spsum.tile([128, NC], F32, tag="s", name="ps_s")
                for k in range(2):
                    nc.tensor.matmul(ps_s[:, :cs], hT6[:, k, :],
                                     echunk[:, k, :],
                                     start=(k == 0), stop=False)
                for p in range(2):
                    for k in range(2):
                        nc.tensor.matmul(
                            ps_s[:, :cs], hT6[:, 2 + p * 2 + k, :],
                            esc[:, p, k, :cs],
                            start=False, stop=(p == 1 and k == 1))
                out_c = work.tile([128, NC], F32, tag="out", name="out_c")
                nc.vector.tensor_copy(out_c[:, :cs], ps_s[:, :cs])
                nc.sync.dma_start(scores_d.ap()[:, ds(c0, cs)], out_c[:, :cs])

    nc.compile()
    return nc


def score_host_inputs(hn_bf, emb, emb_purpose):
    # host: tcw softmax (z = emb @ ep.T is 0.2% of total FLOPs) + hn deltas
    z = emb @ emb_purpose.T                      # [T, 3] f32
    z = z - z.max(axis=1, keepdims=True)
    ez = np.exp(z)
    w = ez / ez.sum(axis=1, keepdims=True)       # tcw

    hn = hn_bf.astype(np.float32)                # [3, B, D]
    h2 = hn[2]
    d0 = hn[0] - h2
    d1 = hn[1] - h2
    # hT6 [128, 6, 128]: stationary tiles [d-part, slot, b]
    hT6 = np.zeros((128, 6, 128), _BF)
    for k in range(2):
        hT6[:, 0 + k, :] = h2.T[k * 128:(k + 1) * 128, :].astype(_BF)
        hT6[:, 2 + k, :] = d0.T[k * 128:(k + 1) * 128, :].astype(_BF)
        hT6[:, 4 + k, :] = d1.T[k * 128:(k + 1) * 128, :].astype(_BF)

    embT = emb.T.astype(_BF)  # [256, 50001]

    base = N_ITEMS // NCORES
    rem = N_ITEMS - base * NCORES
    bounds = []
    s0 = 0
    for c in range(NCORES):
        n = base + (1 if c < rem else 0)
        bounds.append((s0, s0 + n))
        s0 += n

    in_maps = []
    for c in range(NCORES):
        lo, hi = bounds[c]
        n = hi - lo
        eT = np.zeros((128, 2, T_PAD), _BF)
        eT[:, :, :n] = embT[:, lo:hi].reshape(2, 128, n).transpose(1, 0, 2)
        w01 = np.zeros((1, 2, T_PAD), _BF)
        w01[0, :, :n] = w[lo:hi, 0:2].T.astype(_BF)
        in_maps.append({"hT6": hT6, "eT": eT, "w01": w01})
    return in_maps, bounds


# --------------------------------------------------------------------------
# Entry point
# --------------------------------------------------------------------------

_SCAN_NC = None
_SCORE_NC = None


def _get_ncs():
    global _SCAN_NC, _SCORE_NC
    if _SCAN_NC is None:
        _SCAN_NC = build_scan_nc()
    if _SCORE_NC is None:
        _SCORE_NC = build_score_nc()
    return _SCAN_NC, _SCORE_NC


def kernel(seq, emb, emb_purpose, w_ih, w_hh, b_ih, b_hh):
    seq = np.asarray(seq)
    emb = np.asarray(emb, np.float32)
    emb_purpose = np.asarray(emb_purpose, np.float32)
    w_ih = np.asarray(w_ih, np.float32)
    w_hh = np.asarray(w_hh, np.float32)
    b_ih = np.asarray(b_ih, np.float32)
    b_hh = np.asarray(b_hh, np.float32)

    scan_nc, score_nc = _get_ncs()

    scan_ins = scan_host_inputs(seq, emb, emb_purpose, w_ih, w_hh, b_ih, b_hh)
    res1 = run_bass_kernel_spmd(scan_nc, scan_ins, core_ids=list(range(NCORES)))

    hn = np.zeros((3, B, DIM), _BF)
    for c in range(6):
        p, h = CORE_PH[c]
        sl = res1.results[c]["hn_out"].reshape(128, 2, W)
        for k in range(2):
            hn[p, h * W:(h + 1) * W, k * 128:(k + 1) * 128] = sl[:, k, :].T

    score_ins, bounds = score_host_inputs(hn, emb, emb_purpose)
    res2 = run_bass_kernel_spmd(score_nc, score_ins,
                                core_ids=list(range(NCORES)))

    scores = np.empty((B, N_ITEMS), np.float32)
    for c in range(NCORES):
        lo, hi = bounds[c]
        scores[:, lo:hi] = res2.results[c]["scores"][:, : hi - lo]
    return scores


# revision 5
# speedup vs baseline: 1.0753x; 1.0042x over previous
"""Trainium2 Bass kernel for MCPRN (purpose-routed GRU-variant session
recommender). Two SPMD launches on 8 NeuronCores.

Launch 1 (scan): cores 0-5 run (purpose p, batch-half h) PSRU scans as two
  interleaved 32-wide chains. The hidden state is carried as two bf16
  summands (m, an) so the per-step critical path never materializes h:
    h(t) = m(t) + an(t),   m(t) = h(t-1)*(1-a(t)),   an(t) = a(t)*n(t)
  and the gate matmuls take (m, an) as two moving operands accumulating in
  PSUM. x-side gate contributions and biases are precomputed per 4-step
  group (double-buffered PSUM), sliced per step to keep the PE queue fine-
  grained. Concen weights computed on device (softmax/tau, masked, eps-
  clamped) and broadcast across partitions via a DRAM roundtrip.

Launch 2 (score): scores[b,t] = sum_p tcw[t,p] * <hn[p,b,:], emb[t,:]>
  rewritten (sum_p tcw = 1) as  P2 + w0*(P0-P2) + w1*(P1-P2):
  per item chunk, w0/w1 rows are broadcast across partitions with K=1
  ones-matmuls, copied to bf16, the emb chunk is scaled by them on DVE
  (one 4x-mode scalar_tensor_tensor), and all six K=128 matmuls (raw emb
  against hn2, scaled emb against hn_p - hn_2) accumulate into one PSUM
  tile that DMAs straight to DRAM. The softmax tcw itself (50001x3) is
  computed on host; all matmul FLOPs stay on device.
"""

import numpy as np
import ml_dtypes

import concourse.bacc as bacc
import concourse.mybir as mybir
import concourse.tile as tile
from concourse.bass import ts, ds
from concourse.bass_utils import run_bass_kernel_spmd

F32 = mybir.dt.float32
BF16 = mybir.dt.bfloat16
AF = mybir.ActivationFunctionType
OP = mybir.AluOpType

N_ITEMS = 50001
DIM = 256
TAU = 0.1
S = 50
B = 128
EPS = 0.01
W = 64          # batch per scan core
CW = 32         # chain width (two chains per core)
GS = 4          # x-side group size (steps)
SB = S * W      # 3200
NT = SB // 128  # 25
NCORES = 8

CORE_PH = [(0, 0), (0, 1), (1, 0), (1, 1), (2, 0), (2, 1), (0, 0), (0, 1)]

# scoring chunking
T_PAD = 6272            # 49 * 128, per-core padded item count
NC = 512
CHUNK_SIZES = [512] * 12 + [128]
CHUNK_OFFS = np.cumsum([0] + CHUNK_SIZES).tolist()

_BF = ml_dtypes.bfloat16

GROUPS = []
_g0 = 0
while _g0 < S:
    GROUPS.append((_g0, min(GS, S - _g0)))
    _g0 += GS


# --------------------------------------------------------------------------
# Launch 1: scan
# --------------------------------------------------------------------------

def build_scan_nc():
    nc = bacc.Bacc("TRN2", target_bir_lowering=False, debug=False,
                   num_devices=NCORES)

    wiT_d = nc.dram_tensor("wiT", [128, 2, 768], BF16, kind="ExternalInput")
    whT_d = nc.dram_tensor("whT", [128, 2, 768], BF16, kind="ExternalInput")
    xT_d = nc.dram_tensor("xT", [128, 2, SB], BF16, kind="ExternalInput")
    pT_d = nc.dram_tensor("pT", [128, 2, 3], BF16, kind="ExternalInput")
    mask_d = nc.dram_tensor("mask", [128, NT], F32, kind="ExternalInput")
    bri_d = nc.dram_tensor("bri", [1, 512], BF16, kind="ExternalInput")
    bin_d = nc.dram_tensor("bin", [1, 256], BF16, kind="ExternalInput")
    bhn_d = nc.dram_tensor("bhnr", [1, 256], BF16, kind="ExternalInput")
    hn_out = nc.dram_tensor("hn_out", [128, 2 * W], BF16, kind="ExternalOutput")
    cf_lin = nc.dram_tensor("cf_lin", [NT, 128], BF16)

    with tile.TileContext(nc) as tc:
        with (
            tc.tile_pool(name="consts", bufs=1) as consts,
            tc.tile_pool(name="cwp", bufs=1) as cwp,
            tc.tile_pool(name="gx", bufs=2, space="PSUM") as gx,
            tc.tile_pool(name="ghn", bufs=1, space="PSUM") as ghnp,
            tc.tile_pool(name="ew", bufs=3) as ew,
            tc.tile_pool(name="hpool", bufs=3) as hpool,
        ):
            pT = consts.tile_from(pT_d.ap())
            xT = consts.tile_from(xT_d.ap())
            wiT = consts.tile_from(wiT_d.ap())
            whT = consts.tile_from(whT_d.ap())
            mask = consts.tile_from(mask_d.ap())
            bri = consts.tile_from(bri_d.ap())
            bin_ = consts.tile_from(bin_d.ap())
            bhnr = consts.tile_from(bhn_d.ap())
            ones = consts.tile([1, GS * W], BF16)
            nc.vector.memset(ones[:], 1.0)
            # preload the activation tables during the input DMAs
            dummy = consts.tile([1, 1], F32)
            nc.vector.memset(dummy[:], 0.0)
            for fn in (AF.Exp, AF.Sigmoid, AF.Tanh):
                nc.scalar.activation(dummy[:], dummy[:], fn)

            # ---------------- concen -> cf_rep (prologue) ----------------
            # PSUM is fully budgeted for the scan; the concen matmuls borrow
            # the (much larger) x-group "gri" tag buffer before the scan
            # starts using it.
            ps_s = gx.tile([128, NT, 3], F32, tag="gri", name="ps_s")
            for tt in range(NT):
                for k in range(2):
                    nc.tensor.matmul(
                        ps_s[:, tt, :], xT[:, k, ts(tt, 128)], pT[:, k, :],
                        start=(k == 0), stop=(k == 1))
            e_s = cwp.tile([128, NT, 3], F32)
            nc.scalar.activation(e_s[:], ps_s[:], AF.Exp, scale=1.0 / TAU)
            den = cwp.tile([128, NT], F32)
            nc.vector.tensor_reduce(den[:], e_s[:], mybir.AxisListType.X,
                                    OP.add)
            rden = cwp.tile([128, NT], F32)
            nc.vector.reciprocal_approx_fast(rden[:], den[:])
            cnorm = cwp.tile([128, NT, 3], F32)
            nc.vector.tensor_tensor(
                cnorm[:], e_s[:],
                rden[:, :, None].to_broadcast((128, NT, 3)), OP.mult)
            cm = cwp.tile([128, NT, 3], F32)
            nc.vector.tensor_tensor(
                cm[:], cnorm[:],
                mask[:, :, None].to_broadcast((128, NT, 3)), OP.mult)
            ge = cwp.tile([128, NT, 3], F32)
            nc.vector.tensor_scalar(ge[:], cm[:], EPS, None, OP.is_ge)
            cf3 = cwp.tile([128, NT, 3], BF16)
            nc.vector.tensor_tensor(cf3[:], cm[:], ge[:], OP.mult)
            nc.sync.dma_start(cf_lin.ap().rearrange("t p -> p t"),
                              cf3[:, :, 0])
            cf_rep = cwp.tile([128, SB], BF16)
            nc.sync.dma_start(
                cf_rep[:],
                cf_lin.ap().rearrange("t p -> (t p)")[None, :]
                .to_broadcast((128, SB)))

            # ---------------- state ----------------
            m_t, an_t, h_t = {}, {}, {}
            for c in range(2):
                m_t[c] = hpool.tile([128, 2, CW], BF16, tag=f"m{c}",
                                    name=f"m0_{c}")
                nc.vector.memset(m_t[c][:], 0.0)
                h_t[c] = hpool.tile([128, 2, CW], BF16, tag=f"h{c}",
                                    name=f"h0_{c}")
                nc.vector.memset(h_t[c][:], 0.0)
                an_t[c] = hpool.tile([128, 2, CW], BF16, tag=f"an{c}",
                                     name=f"an0_{c}")
                nc.vector.memset(an_t[c][:], 0.0)

            def xstripe(gri, gin, g0, gn, stripes):
                gw = gn * W
                col = ds(g0 * W, gw)
                for s in stripes:
                    if s < 4:
                        j = s
                        nc.tensor.matmul(gri[:, j, :gn, :],
                                         bri[0:1, ts(j, 128)],
                                         ones[0:1, :gw], start=True,
                                         stop=False)
                        for k in range(2):
                            nc.tensor.matmul(
                                gri[:, j, :gn, :], wiT[:, k, ts(j, 128)],
                                xT[:, k, col], start=False, stop=(k == 1))
                    else:
                        j = s - 4
                        nc.tensor.matmul(gin[:, j, :gn, :],
                                         bin_[0:1, ts(j, 128)],
                                         ones[0:1, :gw], start=True,
                                         stop=False)
                        for k in range(2):
                            nc.tensor.matmul(
                                gin[:, j, :gn, :],
                                wiT[:, k, ds(512 + j * 128, 128)],
                                xT[:, k, col], start=False, stop=(k == 1))

            def newgroup():
                gri = gx.tile([128, 4, GS, W], F32, tag="gri", name="gri")
                gin = gx.tile([128, 2, GS, W], F32, tag="gin", name="gin")
                return gri, gin

            def gin_copy(gin, gn):
                gs_sb = ew.tile([128, 2, GS, W], BF16, tag="ginS", name="ginS")
                nc.vector.tensor_copy(gs_sb[:, :, :gn, :], gin[:, :, :gn, :])
                return gs_sb

            cur = newgroup()
            xstripe(*cur, GROUPS[0][0], GROUPS[0][1], range(6))
            cur_sb = gin_copy(cur[1], GROUPS[0][1])
            nxt = newgroup() if len(GROUPS) > 1 else None
            # stripes of the next group, spread over this group's steps
            STRIPE_SCHED = {0: (0, 1), 1: (2, 3), 2: (4,), 3: (5,)}

            for gi, (g0, gn) in enumerate(GROUPS):
                gri, gin = cur
                for tl in range(gn):
                    t = g0 + tl
                    # prefetch stripes of the next x group FIRST so they sit
                    # ahead of the gate matmuls in the PE queue and drain
                    # during this step's elementwise phase
                    if nxt is not None and gi + 1 < len(GROUPS):
                        g0n, gnn = GROUPS[gi + 1]
                        sched = STRIPE_SCHED.get(tl, ())
                        if gn < GS and tl == gn - 1:
                            sched = tuple(s for tl2 in range(tl, GS)
                                          for s in STRIPE_SCHED.get(tl2, ()))
                        xstripe(*nxt, g0n, gnn, sched)
                    for c in range(2):
                        bsl = ds(c * CW, CW)
                        ps_ghn = ghnp.tile([128, 2, CW], F32, tag=f"ghn{c}",
                                           name=f"ghn{c}")
                        # r tiles: m-src first (ready early), then an-src
                        for src in (m_t[c], an_t[c]):
                            for j in range(2):
                                for k in range(2):
                                    nc.tensor.matmul(
                                        gri[:, j, tl, bsl],
                                        whT[:, k, ts(j, 128)],
                                        src[:, k, :], start=False, stop=False,
                                        skip_group_check=True)
                        for src in (m_t[c], an_t[c]):
                            for j in range(2, 4):
                                for k in range(2):
                                    nc.tensor.matmul(
                                        gri[:, j, tl, bsl],
                                        whT[:, k, ts(j, 128)],
                                        src[:, k, :], start=False, stop=False,
                                        skip_group_check=True)
                        for j in range(2):
                            nc.tensor.matmul(
                                ps_ghn[:, j, :], bhnr[0:1, ts(j, 128)],
                                ones[0:1, :CW], start=True, stop=False)
                            for si, src in enumerate((m_t[c], an_t[c])):
                                for k in range(2):
                                    nc.tensor.matmul(
                                        ps_ghn[:, j, :],
                                        whT[:, k, ds(512 + j * 128, 128)],
                                        src[:, k, :],
                                        start=False,
                                        stop=(si == 1 and k == 1))
                        sri = ew.tile([128, 4, CW], BF16, tag=f"sri{c}",
                                      name=f"sri{c}")
                        nc.scalar.activation(sri[:], gri[:, :, tl, bsl],
                                             AF.Sigmoid)
                        u1 = ew.tile([128, 2, CW], BF16, tag=f"u1{c}",
                                     name=f"u1{c}")
                        nc.vector.tensor_tensor(
                            u1[:], ps_ghn[:], sri[:, 0:2, :], OP.mult)
                        u2 = ew.tile([128, 2, CW], BF16, tag=f"u2{c}",
                                     name=f"u2{c}")
                        nc.vector.tensor_tensor(u2[:], u1[:],
                                                cur_sb[:, :, tl, bsl], OP.add)
                        a_t = ew.tile([128, 2, CW], BF16, tag=f"a{c}",
                                      name=f"a{c}")
                        nc.vector.tensor_tensor(
                            a_t[:], sri[:, 2:4, :],
                            cf_rep[:, None, ds(t * W + c * CW, CW)]
                            .to_broadcast((128, 2, CW)), OP.mult)
                        n_t = ew.tile([128, 2, CW], BF16, tag=f"n{c}",
                                      name=f"n{c}")
                        nc.scalar.activation(n_t[:], u2[:], AF.Tanh)
                        # off-critical: q, m (Pool)
                        q_t = ew.tile([128, 2, CW], BF16, tag=f"q{c}",
                                      name=f"q{c}")
                        nc.gpsimd.tensor_scalar(q_t[:], a_t[:], -1.0, 1.0,
                                                OP.mult, OP.add)
                        m_new = hpool.tile([128, 2, CW], BF16, tag=f"m{c}",
                                           name=f"m{c}")
                        nc.gpsimd.tensor_tensor(m_new[:], h_t[c][:], q_t[:],
                                                OP.mult)
                        # critical: an
                        an_new = hpool.tile([128, 2, CW], BF16, tag=f"an{c}",
                                            name=f"an{c}")
                        nc.vector.tensor_tensor(an_new[:], a_t[:], n_t[:],
                                                OP.mult)
                        # off-critical: h materialization (Pool)
                        h_new = hpool.tile([128, 2, CW], BF16, tag=f"h{c}",
                                           name=f"h{c}")
                        nc.gpsimd.tensor_tensor(h_new[:], m_new[:], an_new[:],
                                                OP.add)
                        m_t[c], an_t[c], h_t[c] = m_new, an_new, h_new
                # rotate groups
                if gi + 1 < len(GROUPS):
                    cur = nxt
                    cur_sb = gin_copy(cur[1], GROUPS[gi + 1][1])
                    nxt = newgroup() if gi + 2 < len(GROUPS) else None

            for c in range(2):
                nc.sync.dma_start(
                    hn_out.ap().rearrange("p (k b) -> p k b", k=2)
                    [:, :, ds(c * CW, CW)], h_t[c][:])

    nc.compile()
    return nc


def scan_host_inputs(seq, emb, emb_purpose, w_ih, w_hh, b_ih, b_hh):
    seq = np.asarray(seq)
    xg = emb[seq]                      # [S, B, D] gather (input staging)
    in_maps = []
    for c in range(NCORES):
        p, h = CORE_PH[c]
        sl = slice(h * W, (h + 1) * W)
        xh = xg[:, sl, :]              # [S, W, D]
        xT = np.ascontiguousarray(
            xh.transpose(2, 0, 1).reshape(2, 128, SB).transpose(1, 0, 2))
        wiT = np.ascontiguousarray(
            w_ih[p].T.reshape(2, 128, 768).transpose(1, 0, 2))
        whT = np.ascontiguousarray(
            w_hh[p].T.reshape(2, 128, 768).transpose(1, 0, 2))
        perm = [p, (p + 1) % 3, (p + 2) % 3]
        pT = np.ascontiguousarray(
            emb_purpose[perm].T.reshape(2, 128, 3).transpose(1, 0, 2))
        m = (seq[:, sl] != 0).astype(np.float32).reshape(SB)
        mask = np.ascontiguousarray(m.reshape(NT, 128).T)
        bsum = (b_ih[p] + b_hh[p])[:512]
        in_maps.append({
            "wiT": wiT.astype(_BF), "whT": whT.astype(_BF),
            "xT": xT.astype(_BF),
            "pT": pT.astype(_BF), "mask": mask,
            "bri": bsum[None, :].astype(_BF),
            "bin": b_ih[p][None, 512:].astype(_BF),
            "bhnr": b_hh[p][None, 512:].astype(_BF),
        })
    return in_maps


# --------------------------------------------------------------------------
# Launch 2: scoring
# --------------------------------------------------------------------------

def build_score_nc():
    nc = bacc.Bacc("TRN2", target_bir_lowering=False, debug=False,
                   num_devices=NCORES)

    # hT6: [128, 6, 128] = (d-part, [hn2 k0, hn2 k1, d0 k0, d0 k1, d1 k0,
    #                                d1 k1], b) where d_p = hn_p - hn_2
    hT6_d = nc.dram_tensor("hT6", [128, 6, 128], BF16, kind="ExternalInput")
    eT_d = nc.dram_tensor("eT", [128, 2, T_PAD], BF16, kind="ExternalInput")
    w01_d = nc.dram_tensor("w01", [1, 2, T_PAD], BF16, kind="ExternalInput")
    scores_d = nc.dram_tensor("scores", [128, T_PAD], F32,
                              kind="ExternalOutput")

    QBOUNDS = [0, 1536, 3072, 4608, T_PAD]

    with tile.TileContext(nc) as tc:
        with (
            tc.tile_pool(name="consts", bufs=1) as consts,
            tc.tile_pool(name="wpsum", bufs=2, space="PSUM") as wpsum,
            tc.tile_pool(name="spsum", bufs=2, space="PSUM") as spsum,
            tc.tile_pool(name="work", bufs=4) as work,
        ):
            hT6 = consts.tile_from(hT6_d.ap())
            w01 = consts.tile_from(w01_d.ap())
            onek = consts.tile([1, 128], BF16)
            nc.vector.memset(onek[:], 1.0)
            # quarter the eT load so chunk 0 can start after ~1/4 of the DMA
            eT_q = []
            for q in range(4):
                lo, hi = QBOUNDS[q], QBOUNDS[q + 1]
                eq = consts.tile([128, 2, hi - lo], BF16, name=f"eq{q}")
                nc.sync.dma_start(eq[:], eT_d.ap()[:, :, ds(lo, hi - lo)])
                eT_q.append(eq)

            def esl(c0, cs):
                q = 0
                while QBOUNDS[q + 1] <= c0:
                    q += 1
                assert c0 + cs <= QBOUNDS[q + 1]
                return eT_q[q][:, :, ds(c0 - QBOUNDS[q], cs)]

            for ci, (c0, cs) in enumerate(zip(CHUNK_OFFS[:-1], CHUNK_SIZES)):
                # broadcast w0,w1 rows across partitions (K=1 ones-matmul)
                ps_w = wpsum.tile([128, 2, NC], F32, tag="w", name="ps_w")
                for p in range(2):
                    nc.tensor.matmul(ps_w[:, p, :cs], onek[0:1, :],
                                     w01[0:1, p, ds(c0, cs)],
                                     start=True, stop=True)
                w_sb = work.tile([128, 2, NC], BF16, tag="wsb", name="w_sb")
                nc.scalar.activation(w_sb[:, :, :cs], ps_w[:, :, :cs], AF.Copy)
                # scaled tables: esc[p,k,:] = eT[k,:] * w_p
                # (4 plain TTs, no broadcast APs, so DVE 2x mode applies;
                #  one TT on Pool to balance engines)
                esc = work.tile([128, 2, 2, NC], BF16, tag="esc", name="esc")
                echunk = esl(c0, cs)
                for p in range(2):
                    for k in range(2):
                        eng = nc.gpsimd if (p == 1 and k == 1) else nc.vector
                        eng.tensor_tensor(
                            esc[:, p, k, :cs], echunk[:, k, :],
                            w_sb[:, p, :cs], OP.mult)
                # scores = P2 + w0*dP0 + w1*dP1, all in one PSUM accumulation
                ps_s = spsum.tile([128, NC], F32, tag="s", name="ps_s")
                for k in range(2):
                    nc.tensor.matmul(ps_s[:, :cs], hT6[:, k, :],
                                     echunk[:, k, :],
                                     start=(k == 0), stop=False)
                for p in range(2):
                    for k in range(2):
                        nc.tensor.matmul(
                            ps_s[:, :cs], hT6[:, 2 + p * 2 + k, :],
                            esc[:, p, k, :cs],
                            start=False, stop=(p == 1 and k == 1))
                out_c = work.tile([128, NC], F32, tag="out", name="out_c")
                nc.vector.tensor_copy(out_c[:, :cs], ps_s[:, :cs])
                nc.sync.dma_start(scores_d.ap()[:, ds(c0, cs)], out_c[:, :cs])

    nc.compile()
    return nc


def score_host_inputs(hn_bf, emb, emb_purpose):
    # host: tcw softmax (z = emb @ ep.T is 0.2% of total FLOPs) + hn deltas
    z = emb @ emb_purpose.T                      # [T, 3] f32
    z = z - z.max(axis=1, keepdims=True)
    ez = np.exp(z)
    w = ez / ez.sum(axis=1, keepdims=True)       # tcw

    hn = hn_bf.astype(np.float32)                # [3, B, D]
    h2 = hn[2]
    d0 = hn[0] - h2
    d1 = hn[1] - h2
    # hT6 [128, 6, 128]: stationary tiles [d-part, slot, b]
    hT6 = np.zeros((128, 6, 128), _BF)
    for k in range(2):
        hT6[:, 0 + k, :] = h2.T[k * 128:(k + 1) * 128, :].astype(_BF)
        hT6[:, 2 + k, :] = d0.T[k * 128:(k + 1) * 128, :].astype(_BF)
        hT6[:, 4 + k, :] = d1.T[k * 128:(k + 1) * 128, :].astype(_BF)

    embT = emb.T.astype(_BF)  # [256, 50001]

    base = N_ITEMS // NCORES
    rem = N_ITEMS - base * NCORES
    bounds = []
    s0 = 0
    for c in range(NCORES):
        n = base + (1 if c < rem else 0)
        bounds.append((s0, s0 + n))
        s0 += n

    in_maps = []
    for c in range(NCORES):
        lo, hi = bounds[c]
        n = hi - lo
        eT = np.zeros((128, 2, T_PAD), _BF)
        eT[:, :, :n] = embT[:, lo:hi].reshape(2, 128, n).transpose(1, 0, 2)
        w01 = np.zeros((1, 2, T_PAD), _BF)
        w01[0, :, :n] = w[lo:hi, 0:2].T.astype(_BF)
        in_maps.append({"hT6": hT6, "eT": eT, "w01": w01})
    return in_maps, bounds


# --------------------------------------------------------------------------
# Entry point
# --------------------------------------------------------------------------

_SCAN_NC = None
_SCORE_NC = None


def _get_ncs():
    global _SCAN_NC, _SCORE_NC
    if _SCAN_NC is None:
        _SCAN_NC = build_scan_nc()
    if _SCORE_NC is None:
        _SCORE_NC = build_score_nc()
    return _SCAN_NC, _SCORE_NC


def kernel(seq, emb, emb_purpose, w_ih, w_hh, b_ih, b_hh):
    seq = np.asarray(seq)
    emb = np.asarray(emb, np.float32)
    emb_purpose = np.asarray(emb_purpose, np.float32)
    w_ih = np.asarray(w_ih, np.float32)
    w_hh = np.asarray(w_hh, np.float32)
    b_ih = np.asarray(b_ih, np.float32)
    b_hh = np.asarray(b_hh, np.float32)

    scan_nc, score_nc = _get_ncs()

    scan_ins = scan_host_inputs(seq, emb, emb_purpose, w_ih, w_hh, b_ih, b_hh)
    res1 = run_bass_kernel_spmd(scan_nc, scan_ins, core_ids=list(range(NCORES)))

    hn = np.zeros((3, B, DIM), _BF)
    for c in range(6):
        p, h = CORE_PH[c]
        sl = res1.results[c]["hn_out"].reshape(128, 2, W)
        for k in range(2):
            hn[p, h * W:(h + 1) * W, k * 128:(k + 1) * 128] = sl[:, k, :].T

    score_ins, bounds = score_host_inputs(hn, emb, emb_purpose)
    res2 = run_bass_kernel_spmd(score_nc, score_ins,
                                core_ids=list(range(NCORES)))

    scores = np.empty((B, N_ITEMS), np.float32)
    for c in range(NCORES):
        lo, hi = bounds[c]
        scores[:, lo:hi] = res2.results[c]["scores"][:, : hi - lo]
    return scores


# revision 6
# speedup vs baseline: 1.0779x; 1.0024x over previous
"""Trainium2 Bass kernel for MCPRN (purpose-routed GRU-variant session
recommender). Two SPMD launches on 8 NeuronCores.

Launch 1 (scan): cores 0-5 run (purpose p, batch-half h) PSRU scans as two
  interleaved 32-wide chains. The hidden state is carried as two bf16
  summands (m, an) so the per-step critical path never materializes h:
    h(t) = m(t) + an(t),   m(t) = h(t-1)*(1-a(t)),   an(t) = a(t)*n(t)
  and the gate matmuls take (m, an) as two moving operands accumulating in
  PSUM. x-side gate contributions and biases are precomputed per 4-step
  group (double-buffered PSUM), sliced per step to keep the PE queue fine-
  grained. Concen weights computed on device (softmax/tau, masked, eps-
  clamped) and broadcast across partitions via a DRAM roundtrip.

Launch 2 (score): scores[b,t] = sum_p tcw[t,p] * <hn[p,b,:], emb[t,:]>
  rewritten (sum_p tcw = 1) as  P2 + w0*(P0-P2) + w1*(P1-P2):
  per item chunk, w0/w1 rows are broadcast across partitions with K=1
  ones-matmuls, copied to bf16, the emb chunk is scaled by them on DVE
  (one 4x-mode scalar_tensor_tensor), and all six K=128 matmuls (raw emb
  against hn2, scaled emb against hn_p - hn_2) accumulate into one PSUM
  tile that DMAs straight to DRAM. The softmax tcw itself (50001x3) is
  computed on host; all matmul FLOPs stay on device.
"""

import numpy as np
import ml_dtypes

import concourse.bacc as bacc
import concourse.mybir as mybir
import concourse.tile as tile
from concourse.bass import ts, ds
from concourse.bass_utils import run_bass_kernel_spmd

F32 = mybir.dt.float32
BF16 = mybir.dt.bfloat16
AF = mybir.ActivationFunctionType
OP = mybir.AluOpType

N_ITEMS = 50001
DIM = 256
TAU = 0.1
S = 50
B = 128
EPS = 0.01
W = 64          # batch per scan core
CW = 32         # chain width (two chains per core)
GS = 4          # x-side group size (steps)
SB = S * W      # 3200
NT = SB // 128  # 25
NCORES = 8

CORE_PH = [(0, 0), (0, 1), (1, 0), (1, 1), (2, 0), (2, 1), (0, 0), (0, 1)]

# scoring chunking
T_PAD = 6272            # 49 * 128, per-core padded item count
NC = 512
CHUNK_SIZES = [512] * 12 + [128]
CHUNK_OFFS = np.cumsum([0] + CHUNK_SIZES).tolist()

_BF = ml_dtypes.bfloat16

GROUPS = []
_g0 = 0
while _g0 < S:
    GROUPS.append((_g0, min(GS, S - _g0)))
    _g0 += GS


# --------------------------------------------------------------------------
# Launch 1: scan
# --------------------------------------------------------------------------

def build_scan_nc():
    nc = bacc.Bacc("TRN2", target_bir_lowering=False, debug=False,
                   num_devices=NCORES)

    wiT_d = nc.dram_tensor("wiT", [128, 2, 768], BF16, kind="ExternalInput")
    whT_d = nc.dram_tensor("whT", [128, 2, 768], BF16, kind="ExternalInput")
    xT_d = nc.dram_tensor("xT", [128, 2, SB], BF16, kind="ExternalInput")
    pT_d = nc.dram_tensor("pT", [128, 2, 3], BF16, kind="ExternalInput")
    mask_d = nc.dram_tensor("mask", [128, NT], F32, kind="ExternalInput")
    bri_d = nc.dram_tensor("bri", [1, 512], BF16, kind="ExternalInput")
    bin_d = nc.dram_tensor("bin", [1, 256], BF16, kind="ExternalInput")
    bhn_d = nc.dram_tensor("bhnr", [1, 256], BF16, kind="ExternalInput")
    hn_out = nc.dram_tensor("hn_out", [128, 2 * W], BF16, kind="ExternalOutput")
    cf_lin = nc.dram_tensor("cf_lin", [NT, 128], BF16)

    with tile.TileContext(nc) as tc:
        with (
            tc.tile_pool(name="consts", bufs=1) as consts,
            tc.tile_pool(name="cwp", bufs=1) as cwp,
            tc.tile_pool(name="gx", bufs=2, space="PSUM") as gx,
            tc.tile_pool(name="ghn", bufs=1, space="PSUM") as ghnp,
            tc.tile_pool(name="ew", bufs=3) as ew,
            tc.tile_pool(name="hpool", bufs=3) as hpool,
        ):
            pT = consts.tile_from(pT_d.ap())
            xT = consts.tile_from(xT_d.ap())
            wiT = consts.tile_from(wiT_d.ap())
            whT = consts.tile_from(whT_d.ap())
            mask = consts.tile_from(mask_d.ap())
            bri = consts.tile_from(bri_d.ap())
            bin_ = consts.tile_from(bin_d.ap())
            bhnr = consts.tile_from(bhn_d.ap())
            ones = consts.tile([1, GS * W], BF16)
            nc.vector.memset(ones[:], 1.0)
            # preload the activation tables during the input DMAs
            dummy = consts.tile([1, 1], F32)
            nc.vector.memset(dummy[:], 0.0)
            for fn in (AF.Exp, AF.Sigmoid, AF.Tanh):
                nc.scalar.activation(dummy[:], dummy[:], fn)

            # ---------------- concen -> cf_rep (prologue) ----------------
            # PSUM is fully budgeted for the scan; the concen matmuls borrow
            # the (much larger) x-group "gri" tag buffer before the scan
            # starts using it.
            ps_s = gx.tile([128, NT, 3], F32, tag="gri", name="ps_s")
            for tt in range(NT):
                for k in range(2):
                    nc.tensor.matmul(
                        ps_s[:, tt, :], xT[:, k, ts(tt, 128)], pT[:, k, :],
                        start=(k == 0), stop=(k == 1))
            e_s = cwp.tile([128, NT, 3], F32)
            nc.scalar.activation(e_s[:], ps_s[:], AF.Exp, scale=1.0 / TAU)
            den = cwp.tile([128, NT], F32)
            nc.vector.tensor_reduce(den[:], e_s[:], mybir.AxisListType.X,
                                    OP.add)
            rden = cwp.tile([128, NT], F32)
            nc.vector.reciprocal_approx_fast(rden[:], den[:])
            cnorm = cwp.tile([128, NT, 3], F32)
            nc.vector.tensor_tensor(
                cnorm[:], e_s[:],
                rden[:, :, None].to_broadcast((128, NT, 3)), OP.mult)
            cm = cwp.tile([128, NT, 3], F32)
            nc.vector.tensor_tensor(
                cm[:], cnorm[:],
                mask[:, :, None].to_broadcast((128, NT, 3)), OP.mult)
            ge = cwp.tile([128, NT, 3], F32)
            nc.vector.tensor_scalar(ge[:], cm[:], EPS, None, OP.is_ge)
            cf3 = cwp.tile([128, NT, 3], BF16)
            nc.vector.tensor_tensor(cf3[:], cm[:], ge[:], OP.mult)
            nc.sync.dma_start(cf_lin.ap().rearrange("t p -> p t"),
                              cf3[:, :, 0])
            cf_rep = cwp.tile([128, SB], BF16)
            nc.sync.dma_start(
                cf_rep[:],
                cf_lin.ap().rearrange("t p -> (t p)")[None, :]
                .to_broadcast((128, SB)))

            # ---------------- state ----------------
            m_t, an_t, h_t = {}, {}, {}
            for c in range(2):
                m_t[c] = hpool.tile([128, 2, CW], BF16, tag=f"m{c}",
                                    name=f"m0_{c}")
                nc.vector.memset(m_t[c][:], 0.0)
                h_t[c] = hpool.tile([128, 2, CW], BF16, tag=f"h{c}",
                                    name=f"h0_{c}")
                nc.vector.memset(h_t[c][:], 0.0)
                an_t[c] = hpool.tile([128, 2, CW], BF16, tag=f"an{c}",
                                     name=f"an0_{c}")
                nc.vector.memset(an_t[c][:], 0.0)

            def xstripe(gri, gin, g0, gn, stripes):
                gw = gn * W
                col = ds(g0 * W, gw)
                for s in stripes:
                    if s < 4:
                        j = s
                        nc.tensor.matmul(gri[:, j, :gn, :],
                                         bri[0:1, ts(j, 128)],
                                         ones[0:1, :gw], start=True,
                                         stop=False)
                        for k in range(2):
                            nc.tensor.matmul(
                                gri[:, j, :gn, :], wiT[:, k, ts(j, 128)],
                                xT[:, k, col], start=False, stop=(k == 1))
                    else:
                        j = s - 4
                        nc.tensor.matmul(gin[:, j, :gn, :],
                                         bin_[0:1, ts(j, 128)],
                                         ones[0:1, :gw], start=True,
                                         stop=False)
                        for k in range(2):
                            nc.tensor.matmul(
                                gin[:, j, :gn, :],
                                wiT[:, k, ds(512 + j * 128, 128)],
                                xT[:, k, col], start=False, stop=(k == 1))

            def newgroup():
                gri = gx.tile([128, 4, GS, W], F32, tag="gri", name="gri")
                gin = gx.tile([128, 2, GS, W], F32, tag="gin", name="gin")
                return gri, gin

            def gin_copy(gin, gn):
                gs_sb = ew.tile([128, 2, GS, W], BF16, tag="ginS", name="ginS")
                nc.vector.tensor_copy(gs_sb[:, :, :gn, :], gin[:, :, :gn, :])
                return gs_sb

            cur = newgroup()
            xstripe(*cur, GROUPS[0][0], GROUPS[0][1], range(6))
            cur_sb = gin_copy(cur[1], GROUPS[0][1])
            nxt = newgroup() if len(GROUPS) > 1 else None
            # stripes of the next group, spread over this group's steps
            STRIPE_SCHED = {0: (0, 1), 1: (2, 3), 2: (4,), 3: (5,)}

            for gi, (g0, gn) in enumerate(GROUPS):
                gri, gin = cur
                for tl in range(gn):
                    t = g0 + tl
                    # prefetch stripes of the next x group FIRST so they sit
                    # ahead of the gate matmuls in the PE queue and drain
                    # during this step's elementwise phase
                    if nxt is not None and gi + 1 < len(GROUPS):
                        g0n, gnn = GROUPS[gi + 1]
                        sched = STRIPE_SCHED.get(tl, ())
                        if gn < GS and tl == gn - 1:
                            sched = tuple(s for tl2 in range(tl, GS)
                                          for s in STRIPE_SCHED.get(tl2, ()))
                        xstripe(*nxt, g0n, gnn, sched)
                    for c in range(2):
                        bsl = ds(c * CW, CW)
                        ps_ghn = ghnp.tile([128, 2, CW], F32, tag=f"ghn{c}",
                                           name=f"ghn{c}")
                        # r tiles: m-src first (ready early), then an-src
                        for src in (m_t[c], an_t[c]):
                            for j in range(2):
                                for k in range(2):
                                    nc.tensor.matmul(
                                        gri[:, j, tl, bsl],
                                        whT[:, k, ts(j, 128)],
                                        src[:, k, :], start=False, stop=False,
                                        skip_group_check=True)
                        for src in (m_t[c], an_t[c]):
                            for j in range(2, 4):
                                for k in range(2):
                                    nc.tensor.matmul(
                                        gri[:, j, tl, bsl],
                                        whT[:, k, ts(j, 128)],
                                        src[:, k, :], start=False, stop=False,
                                        skip_group_check=True)
                        for j in range(2):
                            nc.tensor.matmul(
                                ps_ghn[:, j, :], bhnr[0:1, ts(j, 128)],
                                ones[0:1, :CW], start=True, stop=False)
                            for si, src in enumerate((m_t[c], an_t[c])):
                                for k in range(2):
                                    nc.tensor.matmul(
                                        ps_ghn[:, j, :],
                                        whT[:, k, ds(512 + j * 128, 128)],
                                        src[:, k, :],
                                        start=False,
                                        stop=(si == 1 and k == 1))
                        sri = ew.tile([128, 4, CW], BF16, tag=f"sri{c}",
                                      name=f"sri{c}")
                        nc.scalar.activation(sri[:], gri[:, :, tl, bsl],
                                             AF.Sigmoid)
                        u1 = ew.tile([128, 2, CW], BF16, tag=f"u1{c}",
                                     name=f"u1{c}")
                        nc.vector.tensor_tensor(
                            u1[:], ps_ghn[:], sri[:, 0:2, :], OP.mult)
                        u2 = ew.tile([128, 2, CW], BF16, tag=f"u2{c}",
                                     name=f"u2{c}")
                        nc.vector.tensor_tensor(u2[:], u1[:],
                                                cur_sb[:, :, tl, bsl], OP.add)
                        a_t = ew.tile([128, 2, CW], BF16, tag=f"a{c}",
                                      name=f"a{c}")
                        nc.vector.tensor_tensor(
                            a_t[:], sri[:, 2:4, :],
                            cf_rep[:, None, ds(t * W + c * CW, CW)]
                            .to_broadcast((128, 2, CW)), OP.mult)
                        n_t = ew.tile([128, 2, CW], BF16, tag=f"n{c}",
                                      name=f"n{c}")
                        nc.scalar.activation(n_t[:], u2[:], AF.Tanh)
                        # off-critical: q, m (Pool)
                        q_t = ew.tile([128, 2, CW], BF16, tag=f"q{c}",
                                      name=f"q{c}")
                        nc.gpsimd.tensor_scalar(q_t[:], a_t[:], -1.0, 1.0,
                                                OP.mult, OP.add)
                        m_new = hpool.tile([128, 2, CW], BF16, tag=f"m{c}",
                                           name=f"m{c}")
                        nc.gpsimd.tensor_tensor(m_new[:], h_t[c][:], q_t[:],
                                                OP.mult)
                        # critical: an
                        an_new = hpool.tile([128, 2, CW], BF16, tag=f"an{c}",
                                            name=f"an{c}")
                        nc.vector.tensor_tensor(an_new[:], a_t[:], n_t[:],
                                                OP.mult)
                        # off-critical: h materialization (Pool)
                        h_new = hpool.tile([128, 2, CW], BF16, tag=f"h{c}",
                                           name=f"h{c}")
                        nc.gpsimd.tensor_tensor(h_new[:], m_new[:], an_new[:],
                                                OP.add)
                        m_t[c], an_t[c], h_t[c] = m_new, an_new, h_new
                # rotate groups
                if gi + 1 < len(GROUPS):
                    cur = nxt
                    cur_sb = gin_copy(cur[1], GROUPS[gi + 1][1])
                    nxt = newgroup() if gi + 2 < len(GROUPS) else None

            for c in range(2):
                nc.sync.dma_start(
                    hn_out.ap().rearrange("p (k b) -> p k b", k=2)
                    [:, :, ds(c * CW, CW)], h_t[c][:])

    nc.compile()
    return nc


def scan_host_inputs(seq, emb, emb_purpose, w_ih, w_hh, b_ih, b_hh):
    seq = np.asarray(seq)
    xg = emb[seq]                      # [S, B, D] gather (input staging)
    in_maps = []
    for c in range(NCORES):
        p, h = CORE_PH[c]
        sl = slice(h * W, (h + 1) * W)
        xh = xg[:, sl, :]              # [S, W, D]
        xT = np.ascontiguousarray(
            xh.transpose(2, 0, 1).reshape(2, 128, SB).transpose(1, 0, 2))
        wiT = np.ascontiguousarray(
            w_ih[p].T.reshape(2, 128, 768).transpose(1, 0, 2))
        whT = np.ascontiguousarray(
            w_hh[p].T.reshape(2, 128, 768).transpose(1, 0, 2))
        perm = [p, (p + 1) % 3, (p + 2) % 3]
        pT = np.ascontiguousarray(
            emb_purpose[perm].T.reshape(2, 128, 3).transpose(1, 0, 2))
        m = (seq[:, sl] != 0).astype(np.float32).reshape(SB)
        mask = np.ascontiguousarray(m.reshape(NT, 128).T)
        bsum = (b_ih[p] + b_hh[p])[:512]
        in_maps.append({
            "wiT": wiT.astype(_BF), "whT": whT.astype(_BF),
            "xT": xT.astype(_BF),
            "pT": pT.astype(_BF), "mask": mask,
            "bri": bsum[None, :].astype(_BF),
            "bin": b_ih[p][None, 512:].astype(_BF),
            "bhnr": b_hh[p][None, 512:].astype(_BF),
        })
    return in_maps


# --------------------------------------------------------------------------
# Launch 2: scoring
# --------------------------------------------------------------------------

def build_score_nc():
    nc = bacc.Bacc("TRN2", target_bir_lowering=False, debug=False,
                   num_devices=NCORES)

    # hT6: [128, 6, 128] = (d-part, [hn2 k0, hn2 k1, d0 k0, d0 k1, d1 k0,
    #                                d1 k1], b) where d_p = hn_p - hn_2
    hT6_d = nc.dram_tensor("hT6", [128, 6, 128], BF16, kind="ExternalInput")
    eT_d = nc.dram_tensor("eT", [128, 2, T_PAD], BF16, kind="ExternalInput")
    w01_d = nc.dram_tensor("w01", [1, 2, T_PAD], BF16, kind="ExternalInput")
    scores_d = nc.dram_tensor("scores", [128, T_PAD], F32,
                              kind="ExternalOutput")

    QBOUNDS = [0, 1536, 3072, 4608, T_PAD]

    with tile.TileContext(nc) as tc:
        with (
            tc.tile_pool(name="consts", bufs=1) as consts,
            tc.tile_pool(name="wpsum", bufs=3, space="PSUM") as wpsum,
            tc.tile_pool(name="spsum", bufs=2, space="PSUM") as spsum,
            tc.tile_pool(name="work", bufs=4) as work,
        ):
            hT6 = consts.tile_from(hT6_d.ap())
            w01 = consts.tile_from(w01_d.ap())
            onek = consts.tile([1, 128], BF16)
            nc.vector.memset(onek[:], 1.0)
            # quarter the eT load so chunk 0 can start after ~1/4 of the DMA
            eT_q = []
            for q in range(4):
                lo, hi = QBOUNDS[q], QBOUNDS[q + 1]
                eq = consts.tile([128, 2, hi - lo], BF16, name=f"eq{q}")
                nc.sync.dma_start(eq[:], eT_d.ap()[:, :, ds(lo, hi - lo)])
                eT_q.append(eq)

            def esl(c0, cs):
                q = 0
                while QBOUNDS[q + 1] <= c0:
                    q += 1
                assert c0 + cs <= QBOUNDS[q + 1]
                return eT_q[q][:, :, ds(c0 - QBOUNDS[q], cs)]

            for ci, (c0, cs) in enumerate(zip(CHUNK_OFFS[:-1], CHUNK_SIZES)):
                # broadcast w0,w1 rows across partitions (K=1 ones-matmul)
                ps_w = wpsum.tile([128, 2, NC], F32, tag="w", name="ps_w")
                for p in range(2):
                    nc.tensor.matmul(ps_w[:, p, :cs], onek[0:1, :],
                                     w01[0:1, p, ds(c0, cs)],
                                     start=True, stop=True)
                w_sb = work.tile([128, 2, NC], BF16, tag="wsb", name="w_sb")
                nc.scalar.activation(w_sb[:, :, :cs], ps_w[:, :, :cs], AF.Copy)
                # scaled tables: esc[p,k,:] = eT[k,:] * w_p
                # (4 plain TTs, no broadcast APs, so DVE 2x mode applies;
                #  one TT on Pool to balance engines)
                esc = work.tile([128, 2, 2, NC], BF16, tag="esc", name="esc")
                echunk = esl(c0, cs)
                for p in range(2):
                    for k in range(2):
                        eng = nc.gpsimd if (p == 1 and k == 1) else nc.vector
                        eng.tensor_tensor(
                            esc[:, p, k, :cs], echunk[:, k, :],
                            w_sb[:, p, :cs], OP.mult)
                # scores = P2 + w0*dP0 + w1*dP1, all in one PSUM accumulation
                ps_s = spsum.tile([128, NC], F32, tag="s", name="ps_s")
                for k in range(2):
                    nc.tensor.matmul(ps_s[:, :cs], hT6[:, k, :],
                                     echunk[:, k, :],
                                     start=(k == 0), stop=False)
                for p in range(2):
                    for k in range(2):
                        nc.tensor.matmul(
                            ps_s[:, :cs], hT6[:, 2 + p * 2 + k, :],
                            esc[:, p, k, :cs],
                            start=False, stop=(p == 1 and k == 1))
                out_c = work.tile([128, NC], F32, tag="out", name="out_c")
                nc.vector.tensor_copy(out_c[:, :cs], ps_s[:, :cs])
                nc.sync.dma_start(scores_d.ap()[:, ds(c0, cs)], out_c[:, :cs])

    nc.compile()
    return nc


def score_host_inputs(hn_bf, emb, emb_purpose):
    # host: tcw softmax (z = emb @ ep.T is 0.2% of total FLOPs) + hn deltas
    z = emb @ emb_purpose.T                      # [T, 3] f32
    z = z - z.max(axis=1, keepdims=True)
    ez = np.exp(z)
    w = ez / ez.sum(axis=1, keepdims=True)       # tcw

    hn = hn_bf.astype(np.float32)                # [3, B, D]
    h2 = hn[2]
    d0 = hn[0] - h2
    d1 = hn[1] - h2
    # hT6 [128, 6, 128]: stationary tiles [d-part, slot, b]
    hT6 = np.zeros((128, 6, 128), _BF)
    for k in range(2):
        hT6[:, 0 + k, :] = h2.T[k * 128:(k + 1) * 128, :].astype(_BF)
        hT6[:, 2 + k, :] = d0.T[k * 128:(k + 1) * 128, :].astype(_BF)
        hT6[:, 4 + k, :] = d1.T[k * 128:(k + 1) * 128, :].astype(_BF)

    embT = emb.T.astype(_BF)  # [256, 50001]

    base = N_ITEMS // NCORES
    rem = N_ITEMS - base * NCORES
    bounds = []
    s0 = 0
    for c in range(NCORES):
        n = base + (1 if c < rem else 0)
        bounds.append((s0, s0 + n))
        s0 += n

    in_maps = []
    for c in range(NCORES):
        lo, hi = bounds[c]
        n = hi - lo
        eT = np.zeros((128, 2, T_PAD), _BF)
        eT[:, :, :n] = embT[:, lo:hi].reshape(2, 128, n).transpose(1, 0, 2)
        w01 = np.zeros((1, 2, T_PAD), _BF)
        w01[0, :, :n] = w[lo:hi, 0:2].T.astype(_BF)
        in_maps.append({"hT6": hT6, "eT": eT, "w01": w01})
    return in_maps, bounds


# --------------------------------------------------------------------------
# Entry point
# --------------------------------------------------------------------------

_SCAN_NC = None
_SCORE_NC = None


def _get_ncs():
    global _SCAN_NC, _SCORE_NC
    if _SCAN_NC is None:
        _SCAN_NC = build_scan_nc()
    if _SCORE_NC is None:
        _SCORE_NC = build_score_nc()
    return _SCAN_NC, _SCORE_NC


def kernel(seq, emb, emb_purpose, w_ih, w_hh, b_ih, b_hh):
    seq = np.asarray(seq)
    emb = np.asarray(emb, np.float32)
    emb_purpose = np.asarray(emb_purpose, np.float32)
    w_ih = np.asarray(w_ih, np.float32)
    w_hh = np.asarray(w_hh, np.float32)
    b_ih = np.asarray(b_ih, np.float32)
    b_hh = np.asarray(b_hh, np.float32)

    scan_nc, score_nc = _get_ncs()

    scan_ins = scan_host_inputs(seq, emb, emb_purpose, w_ih, w_hh, b_ih, b_hh)
    res1 = run_bass_kernel_spmd(scan_nc, scan_ins, core_ids=list(range(NCORES)))

    hn = np.zeros((3, B, DIM), _BF)
    for c in range(6):
        p, h = CORE_PH[c]
        sl = res1.results[c]["hn_out"].reshape(128, 2, W)
        for k in range(2):
            hn[p, h * W:(h + 1) * W, k * 128:(k + 1) * 128] = sl[:, k, :].T

    score_ins, bounds = score_host_inputs(hn, emb, emb_purpose)
    res2 = run_bass_kernel_spmd(score_nc, score_ins,
                                core_ids=list(range(NCORES)))

    scores = np.empty((B, N_ITEMS), np.float32)
    for c in range(NCORES):
        lo, hi = bounds[c]
        scores[:, lo:hi] = res2.results[c]["scores"][:, : hi - lo]
    return scores


# revision 7
# speedup vs baseline: 1.0801x; 1.0019x over previous
"""Trainium2 Bass kernel for MCPRN (purpose-routed GRU-variant session
recommender). Two SPMD launches on 8 NeuronCores.

Launch 1 (scan): cores 0-5 run (purpose p, batch-half h) PSRU scans as two
  interleaved 32-wide chains. The hidden state is carried as two bf16
  summands (m, an) so the per-step critical path never materializes h:
    h(t) = m(t) + an(t),   m(t) = h(t-1)*(1-a(t)),   an(t) = a(t)*n(t)
  and the gate matmuls take (m, an) as two moving operands accumulating in
  PSUM. x-side gate contributions and biases are precomputed per 4-step
  group (double-buffered PSUM), sliced per step to keep the PE queue fine-
  grained. Concen weights computed on device (softmax/tau, masked, eps-
  clamped) and broadcast across partitions via a DRAM roundtrip.

Launch 2 (score): scores[b,t] = sum_p tcw[t,p] * <hn[p,b,:], emb[t,:]>
  rewritten (sum_p tcw = 1) as  P2 + w0*(P0-P2) + w1*(P1-P2):
  per item chunk, w0/w1 rows are broadcast across partitions with K=1
  ones-matmuls, copied to bf16, the emb chunk is scaled by them on DVE
  (one 4x-mode scalar_tensor_tensor), and all six K=128 matmuls (raw emb
  against hn2, scaled emb against hn_p - hn_2) accumulate into one PSUM
  tile that DMAs straight to DRAM. The softmax tcw itself (50001x3) is
  computed on host; all matmul FLOPs stay on device.
"""

import numpy as np
import ml_dtypes

import concourse.bacc as bacc
import concourse.mybir as mybir
import concourse.tile as tile
from concourse.bass import ts, ds
from concourse.bass_utils import run_bass_kernel_spmd

F32 = mybir.dt.float32
BF16 = mybir.dt.bfloat16
AF = mybir.ActivationFunctionType
OP = mybir.AluOpType

N_ITEMS = 50001
DIM = 256
TAU = 0.1
S = 50
B = 128
EPS = 0.01
W = 64          # batch per scan core
CW = 32         # chain width (two chains per core)
GS = 4          # x-side group size (steps)
SB = S * W      # 3200
NT = SB // 128  # 25
NCORES = 8

CORE_PH = [(0, 0), (0, 1), (1, 0), (1, 1), (2, 0), (2, 1), (0, 0), (0, 1)]

# scoring chunking
T_PAD = 6272            # 49 * 128, per-core padded item count
NC = 512
CHUNK_SIZES = [512] * 12 + [128]
CHUNK_OFFS = np.cumsum([0] + CHUNK_SIZES).tolist()

_BF = ml_dtypes.bfloat16

GROUPS = []
_g0 = 0
while _g0 < S:
    GROUPS.append((_g0, min(GS, S - _g0)))
    _g0 += GS


# --------------------------------------------------------------------------
# Launch 1: scan
# --------------------------------------------------------------------------

def build_scan_nc():
    nc = bacc.Bacc("TRN2", target_bir_lowering=False, debug=False,
                   num_devices=NCORES)

    # inputs are packed into few tensors: HWDGE descriptor generation is
    # exclusive (~625ns per DMA), so fewer DMAs shorten the prologue
    pm_d = nc.dram_tensor("pm", [128, 6 + NT], BF16, kind="ExternalInput")
    brow_d = nc.dram_tensor("brow", [1, 1024], BF16, kind="ExternalInput")
    xT_d = nc.dram_tensor("xT", [128, 2, SB], BF16, kind="ExternalInput")
    wT_d = nc.dram_tensor("wT", [128, 2, 1536], BF16, kind="ExternalInput")
    hn_out = nc.dram_tensor("hn_out", [128, 2 * W], BF16, kind="ExternalOutput")
    cf_lin = nc.dram_tensor("cf_lin", [NT, 128], BF16)

    with tile.TileContext(nc) as tc:
        with (
            tc.tile_pool(name="consts", bufs=1) as consts,
            tc.tile_pool(name="cwp", bufs=1) as cwp,
            tc.tile_pool(name="gx", bufs=2, space="PSUM") as gx,
            tc.tile_pool(name="ghn", bufs=1, space="PSUM") as ghnp,
            tc.tile_pool(name="ew", bufs=3) as ew,
            tc.tile_pool(name="hpool", bufs=3) as hpool,
        ):
            pm = consts.tile_from(pm_d.ap())
            brow = consts.tile_from(brow_d.ap())
            xT = consts.tile_from(xT_d.ap())
            wT = consts.tile_from(wT_d.ap())
            ones = consts.tile([1, GS * W], BF16)
            nc.vector.memset(ones[:], 1.0)
            # preload the activation tables during the input DMAs
            dummy = consts.tile([1, 1], F32)
            nc.vector.memset(dummy[:], 0.0)
            for fn in (AF.Exp, AF.Sigmoid, AF.Tanh):
                nc.scalar.activation(dummy[:], dummy[:], fn)

            # ---------------- concen -> cf_rep (prologue) ----------------
            # PSUM is fully budgeted for the scan; the concen matmuls borrow
            # the (much larger) x-group "gri" tag buffer before the scan
            # starts using it.
            ps_s = gx.tile([128, NT, 3], F32, tag="gri", name="ps_s")
            for tt in range(NT):
                for k in range(2):
                    nc.tensor.matmul(
                        ps_s[:, tt, :], xT[:, k, ts(tt, 128)],
                        pm[:, ds(k * 3, 3)],
                        start=(k == 0), stop=(k == 1))
            e_s = cwp.tile([128, NT, 3], F32)
            nc.scalar.activation(e_s[:], ps_s[:], AF.Exp, scale=1.0 / TAU)
            den = cwp.tile([128, NT], F32)
            nc.vector.tensor_reduce(den[:], e_s[:], mybir.AxisListType.X,
                                    OP.add)
            rden = cwp.tile([128, NT], F32)
            nc.vector.reciprocal_approx_fast(rden[:], den[:])
            cnorm = cwp.tile([128, NT, 3], F32)
            nc.vector.tensor_tensor(
                cnorm[:], e_s[:],
                rden[:, :, None].to_broadcast((128, NT, 3)), OP.mult)
            cm = cwp.tile([128, NT, 3], F32)
            nc.vector.tensor_tensor(
                cm[:], cnorm[:],
                pm[:, ds(6, NT), None].to_broadcast((128, NT, 3)),
                OP.mult)
            ge = cwp.tile([128, NT, 3], F32)
            nc.vector.tensor_scalar(ge[:], cm[:], EPS, None, OP.is_ge)
            cf3 = cwp.tile([128, NT, 3], BF16)
            nc.vector.tensor_tensor(cf3[:], cm[:], ge[:], OP.mult)
            nc.sync.dma_start(cf_lin.ap().rearrange("t p -> p t"),
                              cf3[:, :, 0])
            cf_rep = cwp.tile([128, SB], BF16)
            nc.sync.dma_start(
                cf_rep[:],
                cf_lin.ap().rearrange("t p -> (t p)")[None, :]
                .to_broadcast((128, SB)))

            # ---------------- state ----------------
            m_t, an_t, h_t = {}, {}, {}
            for c in range(2):
                m_t[c] = hpool.tile([128, 2, CW], BF16, tag=f"m{c}",
                                    name=f"m0_{c}")
                nc.vector.memset(m_t[c][:], 0.0)
                h_t[c] = hpool.tile([128, 2, CW], BF16, tag=f"h{c}",
                                    name=f"h0_{c}")
                nc.vector.memset(h_t[c][:], 0.0)
                an_t[c] = hpool.tile([128, 2, CW], BF16, tag=f"an{c}",
                                     name=f"an0_{c}")
                nc.vector.memset(an_t[c][:], 0.0)

            def xstripe(gri, gin, g0, gn, stripes):
                gw = gn * W
                col = ds(g0 * W, gw)
                for s in stripes:
                    if s < 4:
                        j = s
                        nc.tensor.matmul(gri[:, j, :gn, :],
                                         brow[0:1, ts(j, 128)],
                                         ones[0:1, :gw], start=True,
                                         stop=False)
                        for k in range(2):
                            nc.tensor.matmul(
                                gri[:, j, :gn, :], wT[:, k, ts(j, 128)],
                                xT[:, k, col], start=False, stop=(k == 1))
                    else:
                        j = s - 4
                        nc.tensor.matmul(gin[:, j, :gn, :],
                                         brow[0:1, ds(512 + j * 128, 128)],
                                         ones[0:1, :gw], start=True,
                                         stop=False)
                        for k in range(2):
                            nc.tensor.matmul(
                                gin[:, j, :gn, :],
                                wT[:, k, ds(512 + j * 128, 128)],
                                xT[:, k, col], start=False, stop=(k == 1))

            def newgroup():
                gri = gx.tile([128, 4, GS, W], F32, tag="gri", name="gri")
                gin = gx.tile([128, 2, GS, W], F32, tag="gin", name="gin")
                return gri, gin

            def gin_copy(gin, gn):
                gs_sb = ew.tile([128, 2, GS, W], BF16, tag="ginS", name="ginS")
                nc.vector.tensor_copy(gs_sb[:, :, :gn, :], gin[:, :, :gn, :])
                return gs_sb

            cur = newgroup()
            xstripe(*cur, GROUPS[0][0], GROUPS[0][1], range(6))
            cur_sb = gin_copy(cur[1], GROUPS[0][1])
            nxt = newgroup() if len(GROUPS) > 1 else None
            nxt_sb = None
            # stripes of the next group, spread over this group's steps
            STRIPE_SCHED = {0: (0, 1), 1: (2, 3), 2: (4,), 3: (5,)}

            for gi, (g0, gn) in enumerate(GROUPS):
                gri, gin = cur
                for tl in range(gn):
                    t = g0 + tl
                    # prefetch stripes of the next x group FIRST so they sit
                    # ahead of the gate matmuls in the PE queue and drain
                    # during this step's elementwise phase
                    if nxt is not None and gi + 1 < len(GROUPS):
                        g0n, gnn = GROUPS[gi + 1]
                        sched = STRIPE_SCHED.get(tl, ())
                        if gn < GS and tl == gn - 1:
                            sched = tuple(s for tl2 in range(tl, GS)
                                          for s in STRIPE_SCHED.get(tl2, ()))
                        xstripe(*nxt, g0n, gnn, sched)
                    for c in range(2):
                        bsl = ds(c * CW, CW)
                        ps_ghn = ghnp.tile([128, 2, CW], F32, tag=f"ghn{c}",
                                           name=f"ghn{c}")
                        # r tiles: m-src first (ready early), then an-src
                        for src in (m_t[c], an_t[c]):
                            for j in range(2):
                                for k in range(2):
                                    nc.tensor.matmul(
                                        gri[:, j, tl, bsl],
                                        wT[:, k, ds(768 + j * 128, 128)],
                                        src[:, k, :], start=False, stop=False,
                                        skip_group_check=True)
                        for src in (m_t[c], an_t[c]):
                            for j in range(2, 4):
                                for k in range(2):
                                    nc.tensor.matmul(
                                        gri[:, j, tl, bsl],
                                        wT[:, k, ds(768 + j * 128, 128)],
                                        src[:, k, :], start=False, stop=False,
                                        skip_group_check=True)
                        for j in range(2):
                            nc.tensor.matmul(
                                ps_ghn[:, j, :],
                                brow[0:1, ds(768 + j * 128, 128)],
                                ones[0:1, :CW], start=True, stop=False)
                            for si, src in enumerate((m_t[c], an_t[c])):
                                for k in range(2):
                                    nc.tensor.matmul(
                                        ps_ghn[:, j, :],
                                        wT[:, k, ds(1280 + j * 128, 128)],
                                        src[:, k, :],
                                        start=False,
                                        stop=(si == 1 and k == 1))
                        sri = ew.tile([128, 4, CW], BF16, tag=f"sri{c}",
                                      name=f"sri{c}")
                        nc.scalar.activation(sri[:], gri[:, :, tl, bsl],
                                             AF.Sigmoid)
                        u1 = ew.tile([128, 2, CW], BF16, tag=f"u1{c}",
                                     name=f"u1{c}")
                        nc.vector.tensor_tensor(
                            u1[:], ps_ghn[:], sri[:, 0:2, :], OP.mult)
                        u2 = ew.tile([128, 2, CW], BF16, tag=f"u2{c}",
                                     name=f"u2{c}")
                        nc.vector.tensor_tensor(u2[:], u1[:],
                                                cur_sb[:, :, tl, bsl], OP.add)
                        a_t = ew.tile([128, 2, CW], BF16, tag=f"a{c}",
                                      name=f"a{c}")
                        nc.vector.tensor_tensor(
                            a_t[:], sri[:, 2:4, :],
                            cf_rep[:, None, ds(t * W + c * CW, CW)]
                            .to_broadcast((128, 2, CW)), OP.mult)
                        n_t = ew.tile([128, 2, CW], BF16, tag=f"n{c}",
                                      name=f"n{c}")
                        nc.scalar.activation(n_t[:], u2[:], AF.Tanh)
                        # off-critical: q, m (Pool)
                        q_t = ew.tile([128, 2, CW], BF16, tag=f"q{c}",
                                      name=f"q{c}")
                        nc.gpsimd.tensor_scalar(q_t[:], a_t[:], -1.0, 1.0,
                                                OP.mult, OP.add)
                        m_new = hpool.tile([128, 2, CW], BF16, tag=f"m{c}",
                                           name=f"m{c}")
                        nc.gpsimd.tensor_tensor(m_new[:], h_t[c][:], q_t[:],
                                                OP.mult)
                        # critical: an
                        an_new = hpool.tile([128, 2, CW], BF16, tag=f"an{c}",
                                            name=f"an{c}")
                        nc.vector.tensor_tensor(an_new[:], a_t[:], n_t[:],
                                                OP.mult)
                        # off-critical: h materialization (Pool)
                        h_new = hpool.tile([128, 2, CW], BF16, tag=f"h{c}",
                                           name=f"h{c}")
                        nc.gpsimd.tensor_tensor(h_new[:], m_new[:], an_new[:],
                                                OP.add)
                        m_t[c], an_t[c], h_t[c] = m_new, an_new, h_new
                # rotate groups
                if gi + 1 < len(GROUPS):
                    cur = nxt
                    cur_sb = gin_copy(cur[1], GROUPS[gi + 1][1])
                    nxt = newgroup() if gi + 2 < len(GROUPS) else None

            for c in range(2):
                nc.sync.dma_start(
                    hn_out.ap().rearrange("p (k b) -> p k b", k=2)
                    [:, :, ds(c * CW, CW)], h_t[c][:])

    nc.compile()
    return nc


def scan_host_inputs(seq, emb, emb_purpose, w_ih, w_hh, b_ih, b_hh):
    seq = np.asarray(seq)
    xg = emb[seq]                      # [S, B, D] gather (input staging)
    in_maps = []
    for c in range(NCORES):
        p, h = CORE_PH[c]
        sl = slice(h * W, (h + 1) * W)
        xh = xg[:, sl, :]              # [S, W, D]
        xT = np.ascontiguousarray(
            xh.transpose(2, 0, 1).reshape(2, 128, SB).transpose(1, 0, 2))
        wiT = np.ascontiguousarray(
            w_ih[p].T.reshape(2, 128, 768).transpose(1, 0, 2))
        whT = np.ascontiguousarray(
            w_hh[p].T.reshape(2, 128, 768).transpose(1, 0, 2))
        perm = [p, (p + 1) % 3, (p + 2) % 3]
        pT = np.ascontiguousarray(
            emb_purpose[perm].T.reshape(2, 128, 3).transpose(1, 0, 2))
        m = (seq[:, sl] != 0).astype(np.float32).reshape(SB)
        mask = np.ascontiguousarray(m.reshape(NT, 128).T)
        bsum = (b_ih[p] + b_hh[p])[:512]
        pm = np.concatenate(
            [pT.reshape(128, 6), mask], axis=1).astype(_BF)
        brow = np.concatenate(
            [bsum, b_ih[p][512:], b_hh[p][512:]])[None, :].astype(_BF)
        wT = np.concatenate([wiT, whT], axis=2).astype(_BF)
        in_maps.append({
            "pm": pm, "brow": brow, "xT": xT.astype(_BF), "wT": wT,
        })
    return in_maps


# --------------------------------------------------------------------------
# Launch 2: scoring
# --------------------------------------------------------------------------

def build_score_nc():
    nc = bacc.Bacc("TRN2", target_bir_lowering=False, debug=False,
                   num_devices=NCORES)

    # hT6: [128, 6, 128] = (d-part, [hn2 k0, hn2 k1, d0 k0, d0 k1, d1 k0,
    #                                d1 k1], b) where d_p = hn_p - hn_2
    hT6_d = nc.dram_tensor("hT6", [128, 6, 128], BF16, kind="ExternalInput")
    eT_d = nc.dram_tensor("eT", [128, 2, T_PAD], BF16, kind="ExternalInput")
    w01_d = nc.dram_tensor("w01", [1, 2, T_PAD], BF16, kind="ExternalInput")
    scores_d = nc.dram_tensor("scores", [128, T_PAD], F32,
                              kind="ExternalOutput")

    QBOUNDS = [0, 512, 1536, 3072, 4608, T_PAD]

    with tile.TileContext(nc) as tc:
        with (
            tc.tile_pool(name="consts", bufs=1) as consts,
            tc.tile_pool(name="wpsum", bufs=3, space="PSUM") as wpsum,
            tc.tile_pool(name="spsum", bufs=2, space="PSUM") as spsum,
            tc.tile_pool(name="work", bufs=4) as work,
        ):
            hT6 = consts.tile_from(hT6_d.ap())
            w01 = consts.tile_from(w01_d.ap())
            onek = consts.tile([1, 128], BF16)
            nc.vector.memset(onek[:], 1.0)
            # quarter the eT load so chunk 0 can start after ~1/4 of the DMA
            eT_q = []
            for q in range(5):
                lo, hi = QBOUNDS[q], QBOUNDS[q + 1]
                eq = consts.tile([128, 2, hi - lo], BF16, name=f"eq{q}")
                nc.sync.dma_start(eq[:], eT_d.ap()[:, :, ds(lo, hi - lo)])
                eT_q.append(eq)

            def esl(c0, cs):
                q = 0
                while QBOUNDS[q + 1] <= c0:
                    q += 1
                assert c0 + cs <= QBOUNDS[q + 1]
                return eT_q[q][:, :, ds(c0 - QBOUNDS[q], cs)]

            for ci, (c0, cs) in enumerate(zip(CHUNK_OFFS[:-1], CHUNK_SIZES)):
                # broadcast w0,w1 rows across partitions (K=1 ones-matmul)
                ps_w = wpsum.tile([128, 2, NC], F32, tag="w", name="ps_w")
                for p in range(2):
                    nc.tensor.matmul(ps_w[:, p, :cs], onek[0:1, :],
                                     w01[0:1, p, ds(c0, cs)],
                                     start=True, stop=True)
                w_sb = work.tile([128, 2, NC], BF16, tag="wsb", name="w_sb")
                nc.scalar.activation(w_sb[:, :, :cs], ps_w[:, :, :cs], AF.Copy)
                # scaled tables: esc[p,k,:] = eT[k,:] * w_p
                # (4 plain TTs, no broadcast APs, so DVE 2x mode applies;
                #  one TT on Pool to balance engines)
                esc = work.tile([128, 2, 2, NC], BF16, tag="esc", name="esc")
                echunk = esl(c0, cs)
                for p in range(2):
                    for k in range(2):
                        eng = nc.gpsimd if (p == 1 and k == 1) else nc.vector
                        eng.tensor_tensor(
                            esc[:, p, k, :cs], echunk[:, k, :],
                            w_sb[:, p, :cs], OP.mult)
                # scores = P2 + w0*dP0 + w1*dP1, all in one PSUM accumulation
                ps_s = spsum.tile([128, NC], F32, tag="s", name="ps_s")
                for k in range(2):
                    nc.tensor.matmul(ps_s[:, :cs], hT6[:, k, :],
                                     echunk[:, k, :],
                                     start=(k == 0), stop=False)
                for p in range(2):
                    for k in range(2):
                        nc.tensor.matmul(
                            ps_s[:, :cs], hT6[:, 2 + p * 2 + k, :],
                            esc[:, p, k, :cs],
                            start=False, stop=(p == 1 and k == 1))
                out_c = work.tile([128, NC], F32, tag="out", name="out_c")
                nc.vector.tensor_copy(out_c[:, :cs], ps_s[:, :cs])
                nc.sync.dma_start(scores_d.ap()[:, ds(c0, cs)], out_c[:, :cs])

    nc.compile()
    return nc


def score_host_inputs(hn_bf, emb, emb_purpose):
    # host: tcw softmax (z = emb @ ep.T is 0.2% of total FLOPs) + hn deltas
    z = emb @ emb_purpose.T                      # [T, 3] f32
    z = z - z.max(axis=1, keepdims=True)
    ez = np.exp(z)
    w = ez / ez.sum(axis=1, keepdims=True)       # tcw

    hn = hn_bf.astype(np.float32)                # [3, B, D]
    h2 = hn[2]
    d0 = hn[0] - h2
    d1 = hn[1] - h2
    # hT6 [128, 6, 128]: stationary tiles [d-part, slot, b]
    hT6 = np.zeros((128, 6, 128), _BF)
    for k in range(2):
        hT6[:, 0 + k, :] = h2.T[k * 128:(k + 1) * 128, :].astype(_BF)
        hT6[:, 2 + k, :] = d0.T[k * 128:(k + 1) * 128, :].astype(_BF)
        hT6[:, 4 + k, :] = d1.T[k * 128:(k + 1) * 128, :].astype(_BF)

    embT = emb.T.astype(_BF)  # [256, 50001]

    base = N_ITEMS // NCORES
    rem = N_ITEMS - base * NCORES
    bounds = []
    s0 = 0
    for c in range(NCORES):
        n = base + (1 if c < rem else 0)
        bounds.append((s0, s0 + n))
        s0 += n

    in_maps = []
    for c in range(NCORES):
        lo, hi = bounds[c]
        n = hi - lo
        eT = np.zeros((128, 2, T_PAD), _BF)
        eT[:, :, :n] = embT[:, lo:hi].reshape(2, 128, n).transpose(1, 0, 2)
        w01 = np.zeros((1, 2, T_PAD), _BF)
        w01[0, :, :n] = w[lo:hi, 0:2].T.astype(_BF)
        in_maps.append({"hT6": hT6, "eT": eT, "w01": w01})
    return in_maps, bounds


# --------------------------------------------------------------------------
# Entry point
# --------------------------------------------------------------------------

_SCAN_NC = None
_SCORE_NC = None


def _get_ncs():
    global _SCAN_NC, _SCORE_NC
    if _SCAN_NC is None:
        _SCAN_NC = build_scan_nc()
    if _SCORE_NC is None:
        _SCORE_NC = build_score_nc()
    return _SCAN_NC, _SCORE_NC


def kernel(seq, emb, emb_purpose, w_ih, w_hh, b_ih, b_hh):
    seq = np.asarray(seq)
    emb = np.asarray(emb, np.float32)
    emb_purpose = np.asarray(emb_purpose, np.float32)
    w_ih = np.asarray(w_ih, np.float32)
    w_hh = np.asarray(w_hh, np.float32)
    b_ih = np.asarray(b_ih, np.float32)
    b_hh = np.asarray(b_hh, np.float32)

    scan_nc, score_nc = _get_ncs()

    scan_ins = scan_host_inputs(seq, emb, emb_purpose, w_ih, w_hh, b_ih, b_hh)
    res1 = run_bass_kernel_spmd(scan_nc, scan_ins, core_ids=list(range(NCORES)))

    hn = np.zeros((3, B, DIM), _BF)
    for c in range(6):
        p, h = CORE_PH[c]
        sl = res1.results[c]["hn_out"].reshape(128, 2, W)
        for k in range(2):
            hn[p, h * W:(h + 1) * W, k * 128:(k + 1) * 128] = sl[:, k, :].T

    score_ins, bounds = score_host_inputs(hn, emb, emb_purpose)
    res2 = run_bass_kernel_spmd(score_nc, score_ins,
                                core_ids=list(range(NCORES)))

    scores = np.empty((B, N_ITEMS), np.float32)
    for c in range(NCORES):
        lo, hi = bounds[c]
        scores[:, lo:hi] = res2.results[c]["scores"][:, : hi - lo]
    return scores


# revision 8
# speedup vs baseline: 1.0995x; 1.0180x over previous
"""Trainium2 Bass kernel for MCPRN (purpose-routed GRU-variant session
recommender). Two SPMD launches on 8 NeuronCores.

Launch 1 (scan): cores 0-5 run (purpose p, batch-half h) PSRU scans as two
  interleaved 32-wide chains. The hidden state is carried as two bf16
  summands (m, an) so the per-step critical path never materializes h:
    h(t) = m(t) + an(t),   m(t) = h(t-1)*(1-a(t)),   an(t) = a(t)*n(t)
  and the gate matmuls take (m, an) as two moving operands accumulating in
  PSUM. x-side gate contributions and biases are precomputed per 4-step
  group (double-buffered PSUM), sliced per step to keep the PE queue fine-
  grained. Concen weights computed on device (softmax/tau, masked, eps-
  clamped) and broadcast across partitions via a DRAM roundtrip.

Launch 2 (score): scores[b,t] = sum_p tcw[t,p] * <hn[p,b,:], emb[t,:]>
  rewritten (sum_p tcw = 1) as  P2 + w0*(P0-P2) + w1*(P1-P2):
  per item chunk, w0/w1 rows are broadcast across partitions with K=1
  ones-matmuls, copied to bf16, the emb chunk is scaled by them on DVE
  (one 4x-mode scalar_tensor_tensor), and all six K=128 matmuls (raw emb
  against hn2, scaled emb against hn_p - hn_2) accumulate into one PSUM
  tile that DMAs straight to DRAM. The softmax tcw itself (50001x3) is
  computed on host; all matmul FLOPs stay on device.
"""

import numpy as np
import ml_dtypes

import concourse.bacc as bacc
import concourse.mybir as mybir
import concourse.tile as tile
from concourse.bass import ts, ds
from concourse.bass_utils import run_bass_kernel_spmd

F32 = mybir.dt.float32
BF16 = mybir.dt.bfloat16
AF = mybir.ActivationFunctionType
OP = mybir.AluOpType

N_ITEMS = 50001
DIM = 256
TAU = 0.1
S = 50
B = 128
EPS = 0.01
W = 64          # batch per scan core
CW = 32         # chain width (two chains per core)
GS = 4          # x-side group size (steps)
SB = S * W      # 3200
NT = SB // 128  # 25
NCORES = 8

CORE_PH = [(0, 0), (0, 1), (1, 0), (1, 1), (2, 0), (2, 1), (0, 0), (0, 1)]

# scoring chunking
T_PAD = 6272            # 49 * 128, per-core padded item count
NC = 512
CHUNK_SIZES = [512] * 12 + [128]
CHUNK_OFFS = np.cumsum([0] + CHUNK_SIZES).tolist()

_BF = ml_dtypes.bfloat16

GROUPS = []
_g0 = 0
while _g0 < S:
    GROUPS.append((_g0, min(GS, S - _g0)))
    _g0 += GS


# --------------------------------------------------------------------------
# Launch 1: scan
# --------------------------------------------------------------------------

def build_scan_nc():
    nc = bacc.Bacc("TRN2", target_bir_lowering=False, debug=False,
                   num_devices=NCORES)

    # inputs are packed into few tensors: HWDGE descriptor generation is
    # exclusive (~625ns per DMA), so fewer DMAs shorten the prologue
    pm_d = nc.dram_tensor("pm", [128, 6 + NT], BF16, kind="ExternalInput")
    brow_d = nc.dram_tensor("brow", [1, 1024], BF16, kind="ExternalInput")
    xTa_d = nc.dram_tensor("xTa", [128, 2, 512], BF16, kind="ExternalInput")
    xTb_d = nc.dram_tensor("xTb", [128, 2, SB - 512], BF16,
                           kind="ExternalInput")
    wT_d = nc.dram_tensor("wT", [128, 2, 1536], BF16, kind="ExternalInput")
    hn_out = nc.dram_tensor("hn_out", [128, 2 * W], BF16, kind="ExternalOutput")
    cfl_a = nc.dram_tensor("cfl_a", [4, 128], BF16)
    cfl_b = nc.dram_tensor("cfl_b", [NT - 4, 128], BF16)

    with tile.TileContext(nc) as tc:
        with (
            tc.tile_pool(name="consts", bufs=1) as consts,
            tc.tile_pool(name="cwp", bufs=1) as cwp,
            tc.tile_pool(name="gx", bufs=2, space="PSUM") as gx,
            tc.tile_pool(name="ghn", bufs=1, space="PSUM") as ghnp,
            tc.tile_pool(name="ew", bufs=3) as ew,
            tc.tile_pool(name="hpool", bufs=3) as hpool,
        ):
            pm = consts.tile_from(pm_d.ap())
            brow = consts.tile_from(brow_d.ap())
            xTa = consts.tile_from(xTa_d.ap())
            xTb = consts.tile_from(xTb_d.ap())
            wT = consts.tile_from(wT_d.ap())
            ones = consts.tile([1, GS * W], BF16)
            nc.vector.memset(ones[:], 1.0)
            # preload the activation tables during the input DMAs
            dummy = consts.tile([1, 1], F32)
            nc.vector.memset(dummy[:], 0.0)
            for fn in (AF.Exp, AF.Sigmoid, AF.Tanh):
                nc.scalar.activation(dummy[:], dummy[:], fn)

            # ---------------- concen -> cf_rep (prologue) ----------------
            # Two pieces: piece A (steps 0-7) depends only on the small xTa
            # DMA, so the chain's first a(t) unblocks ~4us earlier; piece B
            # computes in the shadow of the first steps.
            def concen_piece(xt, nt, cfl, cfrep_w, tag="gri"):
                ps = gx.tile([128, nt, 3], F32, tag=tag,
                             name=f"ps_{nt}")
                for tt in range(nt):
                    for k in range(2):
                        nc.tensor.matmul(
                            ps[:, tt, :], xt[:, k, ts(tt, 128)],
                            pm[:, ds(k * 3, 3)],
                            start=(k == 0), stop=(k == 1))
                e_s = cwp.tile([128, nt, 3], F32, name=f"e_s{nt}")
                nc.scalar.activation(e_s[:], ps[:], AF.Exp, scale=1.0 / TAU)
                den = cwp.tile([128, nt], F32, name=f"den{nt}")
                nc.vector.tensor_reduce(den[:], e_s[:],
                                        mybir.AxisListType.X, OP.add)
                rden = cwp.tile([128, nt], F32, name=f"rden{nt}")
                nc.vector.reciprocal_approx_fast(rden[:], den[:])
                cm = cwp.tile([128, nt, 3], F32, name=f"cm{nt}")
                nc.vector.tensor_tensor(
                    cm[:], e_s[:],
                    rden[:, :, None].to_broadcast((128, nt, 3)), OP.mult)
                cmm = cwp.tile([128, nt, 3], F32, name=f"cmm{nt}")
                nc.vector.tensor_tensor(
                    cmm[:], cm[:],
                    cfrep_w[:, :, None].to_broadcast((128, nt, 3)), OP.mult)
                ge = cwp.tile([128, nt, 3], F32, name=f"ge{nt}")
                nc.vector.tensor_scalar(ge[:], cmm[:], EPS, None, OP.is_ge)
                cf3 = cwp.tile([128, nt, 3], BF16, name=f"cf3{nt}")
                nc.vector.tensor_tensor(cf3[:], cmm[:], ge[:], OP.mult)
                nc.sync.dma_start(cfl.ap().rearrange("t p -> p t"),
                                  cf3[:, :, 0])
                rep = cwp.tile([128, nt * 128], BF16, name=f"rep{nt}")
                nc.sync.dma_start(
                    rep[:],
                    cfl.ap().rearrange("t p -> (t p)")[None, :]
                    .to_broadcast((128, nt * 128)))
                return rep

            cf_rep_a = concen_piece(xTa, 4, cfl_a, pm[:, ds(6, 4)])
            cf_rep_b = concen_piece(xTb, NT - 4, cfl_b,
                                    pm[:, ds(10, NT - 4)])

            def cf_slice(t, c):
                if t < 8:
                    return cf_rep_a[:, None, ds(t * W + c * CW, CW)]
                return cf_rep_b[:, None, ds((t - 8) * W + c * CW, CW)]

            # ---------------- state ----------------
            m_t, an_t, h_t = {}, {}, {}
            for c in range(2):
                m_t[c] = hpool.tile([128, 2, CW], BF16, tag=f"m{c}",
                                    name=f"m0_{c}")
                nc.vector.memset(m_t[c][:], 0.0)
                h_t[c] = hpool.tile([128, 2, CW], BF16, tag=f"h{c}",
                                    name=f"h0_{c}")
                nc.vector.memset(h_t[c][:], 0.0)
                an_t[c] = hpool.tile([128, 2, CW], BF16, tag=f"an{c}",
                                     name=f"an0_{c}")
                nc.vector.memset(an_t[c][:], 0.0)

            def xstripe(gri, gin, g0, gn, stripes):
                gw = gn * W
                if g0 * W < 512:
                    xT = xTa
                    col = ds(g0 * W, gw)
                else:
                    xT = xTb
                    col = ds(g0 * W - 512, gw)
                for s in stripes:
                    if s < 4:
                        j = s
                        nc.tensor.matmul(gri[:, j, :gn, :],
                                         brow[0:1, ts(j, 128)],
                                         ones[0:1, :gw], start=True,
                                         stop=False)
                        for k in range(2):
                            nc.tensor.matmul(
                                gri[:, j, :gn, :], wT[:, k, ts(j, 128)],
                                xT[:, k, col], start=False, stop=(k == 1))
                    else:
                        j = s - 4
                        nc.tensor.matmul(gin[:, j, :gn, :],
                                         brow[0:1, ds(512 + j * 128, 128)],
                                         ones[0:1, :gw], start=True,
                                         stop=False)
                        for k in range(2):
                            nc.tensor.matmul(
                                gin[:, j, :gn, :],
                                wT[:, k, ds(512 + j * 128, 128)],
                                xT[:, k, col], start=False, stop=(k == 1))

            def newgroup():
                gri = gx.tile([128, 4, GS, W], F32, tag="gri", name="gri")
                gin = gx.tile([128, 2, GS, W], F32, tag="gin", name="gin")
                return gri, gin

            def gin_copy(gin, gn):
                gs_sb = ew.tile([128, 2, GS, W], BF16, tag="ginS", name="ginS")
                nc.vector.tensor_copy(gs_sb[:, :, :gn, :], gin[:, :, :gn, :])
                return gs_sb

            cur = newgroup()
            xstripe(*cur, GROUPS[0][0], GROUPS[0][1], range(6))
            cur_sb = gin_copy(cur[1], GROUPS[0][1])
            nxt = newgroup() if len(GROUPS) > 1 else None
            nxt_sb = None
            # stripes of the next group, spread over this group's steps
            STRIPE_SCHED = {0: (0, 1), 1: (2, 3), 2: (4,), 3: (5,)}

            for gi, (g0, gn) in enumerate(GROUPS):
                gri, gin = cur
                for tl in range(gn):
                    t = g0 + tl
                    # prefetch stripes of the next x group FIRST so they sit
                    # ahead of the gate matmuls in the PE queue and drain
                    # during this step's elementwise phase
                    if nxt is not None and gi + 1 < len(GROUPS):
                        g0n, gnn = GROUPS[gi + 1]
                        sched = STRIPE_SCHED.get(tl, ())
                        if gn < GS and tl == gn - 1:
                            sched = tuple(s for tl2 in range(tl, GS)
                                          for s in STRIPE_SCHED.get(tl2, ()))
                        xstripe(*nxt, g0n, gnn, sched)
                    for c in range(2):
                        bsl = ds(c * CW, CW)
                        ps_ghn = ghnp.tile([128, 2, CW], F32, tag=f"ghn{c}",
                                           name=f"ghn{c}")
                        # r tiles: m-src first (ready early), then an-src
                        for src in (m_t[c], an_t[c]):
                            for j in range(2):
                                for k in range(2):
                                    nc.tensor.matmul(
                                        gri[:, j, tl, bsl],
                                        wT[:, k, ds(768 + j * 128, 128)],
                                        src[:, k, :], start=False, stop=False,
                                        skip_group_check=True)
                        for src in (m_t[c], an_t[c]):
                            for j in range(2, 4):
                                for k in range(2):
                                    nc.tensor.matmul(
                                        gri[:, j, tl, bsl],
                                        wT[:, k, ds(768 + j * 128, 128)],
                                        src[:, k, :], start=False, stop=False,
                                        skip_group_check=True)
                        for j in range(2):
                            nc.tensor.matmul(
                                ps_ghn[:, j, :],
                                brow[0:1, ds(768 + j * 128, 128)],
                                ones[0:1, :CW], start=True, stop=False)
                            for si, src in enumerate((m_t[c], an_t[c])):
                                for k in range(2):
                                    nc.tensor.matmul(
                                        ps_ghn[:, j, :],
                                        wT[:, k, ds(1280 + j * 128, 128)],
                                        src[:, k, :],
                                        start=False,
                                        stop=(si == 1 and k == 1))
                        sri = ew.tile([128, 4, CW], BF16, tag=f"sri{c}",
                                      name=f"sri{c}")
                        nc.scalar.activation(sri[:], gri[:, :, tl, bsl],
                                             AF.Sigmoid)
                        u1 = ew.tile([128, 2, CW], BF16, tag=f"u1{c}",
                                     name=f"u1{c}")
                        nc.vector.tensor_tensor(
                            u1[:], ps_ghn[:], sri[:, 0:2, :], OP.mult)
                        u2 = ew.tile([128, 2, CW], BF16, tag=f"u2{c}",
                                     name=f"u2{c}")
                        nc.vector.tensor_tensor(u2[:], u1[:],
                                                cur_sb[:, :, tl, bsl], OP.add)
                        a_t = ew.tile([128, 2, CW], BF16, tag=f"a{c}",
                                      name=f"a{c}")
                        nc.vector.tensor_tensor(
                            a_t[:], sri[:, 2:4, :],
                            cf_slice(t, c).to_broadcast((128, 2, CW)),
                            OP.mult)
                        n_t = ew.tile([128, 2, CW], BF16, tag=f"n{c}",
                                      name=f"n{c}")
                        nc.scalar.activation(n_t[:], u2[:], AF.Tanh)
                        # off-critical: q, m (Pool)
                        q_t = ew.tile([128, 2, CW], BF16, tag=f"q{c}",
                                      name=f"q{c}")
                        nc.gpsimd.tensor_scalar(q_t[:], a_t[:], -1.0, 1.0,
                                                OP.mult, OP.add)
                        m_new = hpool.tile([128, 2, CW], BF16, tag=f"m{c}",
                                           name=f"m{c}")
                        nc.gpsimd.tensor_tensor(m_new[:], h_t[c][:], q_t[:],
                                                OP.mult)
                        # critical: an
                        an_new = hpool.tile([128, 2, CW], BF16, tag=f"an{c}",
                                            name=f"an{c}")
                        nc.vector.tensor_tensor(an_new[:], a_t[:], n_t[:],
                                                OP.mult)
                        # off-critical: h materialization (Pool)
                        h_new = hpool.tile([128, 2, CW], BF16, tag=f"h{c}",
                                           name=f"h{c}")
                        nc.gpsimd.tensor_tensor(h_new[:], m_new[:], an_new[:],
                                                OP.add)
                        m_t[c], an_t[c], h_t[c] = m_new, an_new, h_new
                # rotate groups
                if gi + 1 < len(GROUPS):
                    cur = nxt
                    cur_sb = gin_copy(cur[1], GROUPS[gi + 1][1])
                    nxt = newgroup() if gi + 2 < len(GROUPS) else None

            for c in range(2):
                nc.sync.dma_start(
                    hn_out.ap().rearrange("p (k b) -> p k b", k=2)
                    [:, :, ds(c * CW, CW)], h_t[c][:])

    nc.compile()
    return nc


def scan_host_inputs(seq, emb, emb_purpose, w_ih, w_hh, b_ih, b_hh):
    seq = np.asarray(seq)
    xg = emb[seq]                      # [S, B, D] gather (input staging)
    in_maps = []
    for c in range(NCORES):
        p, h = CORE_PH[c]
        sl = slice(h * W, (h + 1) * W)
        xh = xg[:, sl, :]              # [S, W, D]
        xT = np.ascontiguousarray(
            xh.transpose(2, 0, 1).reshape(2, 128, SB).transpose(1, 0, 2))
        wiT = np.ascontiguousarray(
            w_ih[p].T.reshape(2, 128, 768).transpose(1, 0, 2))
        whT = np.ascontiguousarray(
            w_hh[p].T.reshape(2, 128, 768).transpose(1, 0, 2))
        perm = [p, (p + 1) % 3, (p + 2) % 3]
        pT = np.ascontiguousarray(
            emb_purpose[perm].T.reshape(2, 128, 3).transpose(1, 0, 2))
        m = (seq[:, sl] != 0).astype(np.float32).reshape(SB)
        mask = np.ascontiguousarray(m.reshape(NT, 128).T)
        bsum = (b_ih[p] + b_hh[p])[:512]
        pm = np.concatenate(
            [pT.reshape(128, 6), mask], axis=1).astype(_BF)
        xTbf = xT.astype(_BF)
        brow = np.concatenate(
            [bsum, b_ih[p][512:], b_hh[p][512:]])[None, :].astype(_BF)
        wT = np.concatenate([wiT, whT], axis=2).astype(_BF)
        in_maps.append({
            "pm": pm, "brow": brow, "xTa": np.ascontiguousarray(xTbf[:, :, :512]),
            "xTb": np.ascontiguousarray(xTbf[:, :, 512:]), "wT": wT,
        })
    return in_maps


# --------------------------------------------------------------------------
# Launch 2: scoring
# --------------------------------------------------------------------------

def build_score_nc():
    nc = bacc.Bacc("TRN2", target_bir_lowering=False, debug=False,
                   num_devices=NCORES)

    # hT6: [128, 6, 128] = (d-part, [hn2 k0, hn2 k1, d0 k0, d0 k1, d1 k0,
    #                                d1 k1], b) where d_p = hn_p - hn_2
    hT6_d = nc.dram_tensor("hT6", [128, 6, 128], BF16, kind="ExternalInput")
    eT_d = nc.dram_tensor("eT", [128, 2, T_PAD], BF16, kind="ExternalInput")
    w01_d = nc.dram_tensor("w01", [1, 2, T_PAD], BF16, kind="ExternalInput")
    scores_d = nc.dram_tensor("scores", [128, T_PAD], F32,
                              kind="ExternalOutput")

    QBOUNDS = [0, 512, 1536, 3072, 4608, T_PAD]

    with tile.TileContext(nc) as tc:
        with (
            tc.tile_pool(name="consts", bufs=1) as consts,
            tc.tile_pool(name="wpsum", bufs=3, space="PSUM") as wpsum,
            tc.tile_pool(name="spsum", bufs=2, space="PSUM") as spsum,
            tc.tile_pool(name="work", bufs=4) as work,
        ):
            hT6 = consts.tile_from(hT6_d.ap())
            w01 = consts.tile_from(w01_d.ap())
            onek = consts.tile([1, 128], BF16)
            nc.vector.memset(onek[:], 1.0)
            # quarter the eT load so chunk 0 can start after ~1/4 of the DMA
            eT_q = []
            for q in range(5):
                lo, hi = QBOUNDS[q], QBOUNDS[q + 1]
                eq = consts.tile([128, 2, hi - lo], BF16, name=f"eq{q}")
                nc.sync.dma_start(eq[:], eT_d.ap()[:, :, ds(lo, hi - lo)])
                eT_q.append(eq)

            def esl(c0, cs):
                q = 0
                while QBOUNDS[q + 1] <= c0:
                    q += 1
                assert c0 + cs <= QBOUNDS[q + 1]
                return eT_q[q][:, :, ds(c0 - QBOUNDS[q], cs)]

            for ci, (c0, cs) in enumerate(zip(CHUNK_OFFS[:-1], CHUNK_SIZES)):
                # broadcast w0,w1 rows across partitions (K=1 ones-matmul)
                ps_w = wpsum.tile([128, 2, NC], F32, tag="w", name="ps_w")
                for p in range(2):
                    nc.tensor.matmul(ps_w[:, p, :cs], onek[0:1, :],
                                     w01[0:1, p, ds(c0, cs)],
                                     start=True, stop=True)
                w_sb = work.tile([128, 2, NC], BF16, tag="wsb", name="w_sb")
                nc.scalar.activation(w_sb[:, :, :cs], ps_w[:, :, :cs], AF.Copy)
                # scaled tables: esc[p,k,:] = eT[k,:] * w_p
                # (4 plain TTs, no broadcast APs, so DVE 2x mode applies;
                #  one TT on Pool to balance engines)
                esc = work.tile([128, 2, 2, NC], BF16, tag="esc", name="esc")
                echunk = esl(c0, cs)
                for p in range(2):
                    for k in range(2):
                        eng = nc.gpsimd if (p == 1 and k == 1) else nc.vector
                        eng.tensor_tensor(
                            esc[:, p, k, :cs], echunk[:, k, :],
                            w_sb[:, p, :cs], OP.mult)
                # scores = P2 + w0*dP0 + w1*dP1, all in one PSUM accumulation
                ps_s = spsum.tile([128, NC], F32, tag="s", name="ps_s")
                for k in range(2):
                    nc.tensor.matmul(ps_s[:, :cs], hT6[:, k, :],
                                     echunk[:, k, :],
                                     start=(k == 0), stop=False)
                for p in range(2):
                    for k in range(2):
                        nc.tensor.matmul(
                            ps_s[:, :cs], hT6[:, 2 + p * 2 + k, :],
                            esc[:, p, k, :cs],
                            start=False, stop=(p == 1 and k == 1))
                out_c = work.tile([128, NC], F32, tag="out", name="out_c")
                nc.vector.tensor_copy(out_c[:, :cs], ps_s[:, :cs])
                nc.sync.dma_start(scores_d.ap()[:, ds(c0, cs)], out_c[:, :cs])

    nc.compile()
    return nc


def score_host_inputs(hn_bf, emb, emb_purpose):
    # host: tcw softmax (z = emb @ ep.T is 0.2% of total FLOPs) + hn deltas
    z = emb @ emb_purpose.T                      # [T, 3] f32
    z = z - z.max(axis=1, keepdims=True)
    ez = np.exp(z)
    w = ez / ez.sum(axis=1, keepdims=True)       # tcw

    hn = hn_bf.astype(np.float32)                # [3, B, D]
    h2 = hn[2]
    d0 = hn[0] - h2
    d1 = hn[1] - h2
    # hT6 [128, 6, 128]: stationary tiles [d-part, slot, b]
    hT6 = np.zeros((128, 6, 128), _BF)
    for k in range(2):
        hT6[:, 0 + k, :] = h2.T[k * 128:(k + 1) * 128, :].astype(_BF)
        hT6[:, 2 + k, :] = d0.T[k * 128:(k + 1) * 128, :].astype(_BF)
        hT6[:, 4 + k, :] = d1.T[k * 128:(k + 1) * 128, :].astype(_BF)

    embT = emb.T.astype(_BF)  # [256, 50001]

    base = N_ITEMS // NCORES
    rem = N_ITEMS - base * NCORES
    bounds = []
    s0 = 0
    for c in range(NCORES):
        n = base + (1 if c < rem else 0)
        bounds.append((s0, s0 + n))
        s0 += n

    in_maps = []
    for c in range(NCORES):
        lo, hi = bounds[c]
        n = hi - lo
        eT = np.zeros((128, 2, T_PAD), _BF)
        eT[:, :, :n] = embT[:, lo:hi].reshape(2, 128, n).transpose(1, 0, 2)
        w01 = np.zeros((1, 2, T_PAD), _BF)
        w01[0, :, :n] = w[lo:hi, 0:2].T.astype(_BF)
        in_maps.append({"hT6": hT6, "eT": eT, "w01": w01})
    return in_maps, bounds


# --------------------------------------------------------------------------
# Entry point
# --------------------------------------------------------------------------

_SCAN_NC = None
_SCORE_NC = None


def _get_ncs():
    global _SCAN_NC, _SCORE_NC
    if _SCAN_NC is None:
        _SCAN_NC = build_scan_nc()
    if _SCORE_NC is None:
        _SCORE_NC = build_score_nc()
    return _SCAN_NC, _SCORE_NC


def kernel(seq, emb, emb_purpose, w_ih, w_hh, b_ih, b_hh):
    seq = np.asarray(seq)
    emb = np.asarray(emb, np.float32)
    emb_purpose = np.asarray(emb_purpose, np.float32)
    w_ih = np.asarray(w_ih, np.float32)
    w_hh = np.asarray(w_hh, np.float32)
    b_ih = np.asarray(b_ih, np.float32)
    b_hh = np.asarray(b_hh, np.float32)

    scan_nc, score_nc = _get_ncs()

    scan_ins = scan_host_inputs(seq, emb, emb_purpose, w_ih, w_hh, b_ih, b_hh)
    res1 = run_bass_kernel_spmd(scan_nc, scan_ins, core_ids=list(range(NCORES)))

    hn = np.zeros((3, B, DIM), _BF)
    for c in range(6):
        p, h = CORE_PH[c]
        sl = res1.results[c]["hn_out"].reshape(128, 2, W)
        for k in range(2):
            hn[p, h * W:(h + 1) * W, k * 128:(k + 1) * 128] = sl[:, k, :].T

    score_ins, bounds = score_host_inputs(hn, emb, emb_purpose)
    res2 = run_bass_kernel_spmd(score_nc, score_ins,
                                core_ids=list(range(NCORES)))

    scores = np.empty((B, N_ITEMS), np.float32)
    for c in range(NCORES):
        lo, hi = bounds[c]
        scores[:, lo:hi] = res2.results[c]["scores"][:, : hi - lo]
    return scores


# revision 9
# speedup vs baseline: 1.1008x; 1.0012x over previous
"""Trainium2 Bass kernel for MCPRN (purpose-routed GRU-variant session
recommender). Two SPMD launches on 8 NeuronCores.

Launch 1 (scan): cores 0-5 run (purpose p, batch-half h) PSRU scans as two
  interleaved 32-wide chains. The hidden state is carried as two bf16
  summands (m, an) so the per-step critical path never materializes h:
    h(t) = m(t) + an(t),   m(t) = h(t-1)*(1-a(t)),   an(t) = a(t)*n(t)
  and the gate matmuls take (m, an) as two moving operands accumulating in
  PSUM. x-side gate contributions and biases are precomputed per 4-step
  group (double-buffered PSUM), sliced per step to keep the PE queue fine-
  grained. Concen weights computed on device (softmax/tau, masked, eps-
  clamped) and broadcast across partitions via a DRAM roundtrip.

Launch 2 (score): scores[b,t] = sum_p tcw[t,p] * <hn[p,b,:], emb[t,:]>
  rewritten (sum_p tcw = 1) as  P2 + w0*(P0-P2) + w1*(P1-P2):
  per item chunk, w0/w1 rows are broadcast across partitions with K=1
  ones-matmuls, copied to bf16, the emb chunk is scaled by them on DVE
  (one 4x-mode scalar_tensor_tensor), and all six K=128 matmuls (raw emb
  against hn2, scaled emb against hn_p - hn_2) accumulate into one PSUM
  tile that DMAs straight to DRAM. The softmax tcw itself (50001x3) is
  computed on host; all matmul FLOPs stay on device.
"""

import numpy as np
import ml_dtypes

import concourse.bacc as bacc
import concourse.mybir as mybir
import concourse.tile as tile
from concourse.bass import ts, ds
from concourse.bass_utils import run_bass_kernel_spmd

F32 = mybir.dt.float32
BF16 = mybir.dt.bfloat16
AF = mybir.ActivationFunctionType
OP = mybir.AluOpType

N_ITEMS = 50001
DIM = 256
TAU = 0.1
S = 50
B = 128
EPS = 0.01
W = 64          # batch per scan core
CW = 32         # chain width (two chains per core)
GS = 4          # x-side group size (steps)
SB = S * W      # 3200
NT = SB // 128  # 25
NCORES = 8

CORE_PH = [(0, 0), (0, 1), (1, 0), (1, 1), (2, 0), (2, 1), (0, 0), (0, 1)]

# scoring chunking
T_PAD = 6272            # 49 * 128, per-core padded item count
NC = 512
CHUNK_SIZES = [512] * 12 + [128]
CHUNK_OFFS = np.cumsum([0] + CHUNK_SIZES).tolist()

_BF = ml_dtypes.bfloat16

GROUPS = []
_g0 = 0
while _g0 < S:
    GROUPS.append((_g0, min(GS, S - _g0)))
    _g0 += GS


# --------------------------------------------------------------------------
# Launch 1: scan
# --------------------------------------------------------------------------

def build_scan_nc():
    nc = bacc.Bacc("TRN2", target_bir_lowering=False, debug=False,
                   num_devices=NCORES)

    # inputs are packed into few tensors: HWDGE descriptor generation is
    # exclusive (~625ns per DMA), so fewer DMAs shorten the prologue
    pm_d = nc.dram_tensor("pm", [128, 6 + NT], BF16, kind="ExternalInput")
    brow_d = nc.dram_tensor("brow", [1, 1024], BF16, kind="ExternalInput")
    xTa_d = nc.dram_tensor("xTa", [128, 2, 512], BF16, kind="ExternalInput")
    xTb_d = nc.dram_tensor("xTb", [128, 2, SB - 512], BF16,
                           kind="ExternalInput")
    wT_d = nc.dram_tensor("wT", [128, 2, 1536], BF16, kind="ExternalInput")
    hn_out = nc.dram_tensor("hn_out", [128, 2 * W], BF16, kind="ExternalOutput")
    cfl_a = nc.dram_tensor("cfl_a", [4, 128], BF16)
    cfl_b = nc.dram_tensor("cfl_b", [NT - 4, 128], BF16)

    with tile.TileContext(nc) as tc:
        with (
            tc.tile_pool(name="consts", bufs=1) as consts,
            tc.tile_pool(name="cwp", bufs=1) as cwp,
            tc.tile_pool(name="gx", bufs=2, space="PSUM") as gx,
            tc.tile_pool(name="ghn", bufs=1, space="PSUM") as ghnp,
            tc.tile_pool(name="ew", bufs=3) as ew,
            tc.tile_pool(name="hpool", bufs=3) as hpool,
        ):
            pm = consts.tile_from(pm_d.ap())
            brow = consts.tile_from(brow_d.ap())
            xTa = consts.tile_from(xTa_d.ap())
            wT = consts.tile_from(wT_d.ap())
            xTb = consts.tile_from(xTb_d.ap())
            ones = consts.tile([1, GS * W], BF16)
            nc.vector.memset(ones[:], 1.0)
            # preload the activation tables during the input DMAs
            dummy = consts.tile([1, 1], F32)
            nc.vector.memset(dummy[:], 0.0)
            for fn in (AF.Exp, AF.Sigmoid, AF.Tanh):
                nc.scalar.activation(dummy[:], dummy[:], fn)

            # ---------------- concen -> cf_rep (prologue) ----------------
            # Two pieces: piece A (steps 0-7) depends only on the small xTa
            # DMA, so the chain's first a(t) unblocks ~4us earlier; piece B
            # computes in the shadow of the first steps.
            def concen_piece(xt, nt, cfl, cfrep_w, tag="gri"):
                ps = gx.tile([128, nt, 3], F32, tag=tag,
                             name=f"ps_{nt}")
                for tt in range(nt):
                    for k in range(2):
                        nc.tensor.matmul(
                            ps[:, tt, :], xt[:, k, ts(tt, 128)],
                            pm[:, ds(k * 3, 3)],
                            start=(k == 0), stop=(k == 1))
                e_s = cwp.tile([128, nt, 3], F32, name=f"e_s{nt}")
                nc.scalar.activation(e_s[:], ps[:], AF.Exp, scale=1.0 / TAU)
                den = cwp.tile([128, nt], F32, name=f"den{nt}")
                nc.vector.tensor_reduce(den[:], e_s[:],
                                        mybir.AxisListType.X, OP.add)
                rden = cwp.tile([128, nt], F32, name=f"rden{nt}")
                nc.vector.reciprocal_approx_fast(rden[:], den[:])
                cm = cwp.tile([128, nt, 3], F32, name=f"cm{nt}")
                nc.vector.tensor_tensor(
                    cm[:], e_s[:],
                    rden[:, :, None].to_broadcast((128, nt, 3)), OP.mult)
                cmm = cwp.tile([128, nt, 3], F32, name=f"cmm{nt}")
                nc.vector.tensor_tensor(
                    cmm[:], cm[:],
                    cfrep_w[:, :, None].to_broadcast((128, nt, 3)), OP.mult)
                ge = cwp.tile([128, nt, 3], F32, name=f"ge{nt}")
                nc.vector.tensor_scalar(ge[:], cmm[:], EPS, None, OP.is_ge)
                cf3 = cwp.tile([128, nt, 3], BF16, name=f"cf3{nt}")
                nc.vector.tensor_tensor(cf3[:], cmm[:], ge[:], OP.mult)
                nc.sync.dma_start(cfl.ap().rearrange("t p -> p t"),
                                  cf3[:, :, 0])
                rep = cwp.tile([128, nt * 128], BF16, name=f"rep{nt}")
                nc.sync.dma_start(
                    rep[:],
                    cfl.ap().rearrange("t p -> (t p)")[None, :]
                    .to_broadcast((128, nt * 128)))
                return rep

            cf_rep_a = concen_piece(xTa, 4, cfl_a, pm[:, ds(6, 4)])
            cf_rep_b = concen_piece(xTb, NT - 4, cfl_b,
                                    pm[:, ds(10, NT - 4)])

            def cf_slice(t, c):
                if t < 8:
                    return cf_rep_a[:, None, ds(t * W + c * CW, CW)]
                return cf_rep_b[:, None, ds((t - 8) * W + c * CW, CW)]

            # ---------------- state ----------------
            m_t, an_t, h_t = {}, {}, {}
            for c in range(2):
                m_t[c] = hpool.tile([128, 2, CW], BF16, tag=f"m{c}",
                                    name=f"m0_{c}")
                nc.vector.memset(m_t[c][:], 0.0)
                h_t[c] = hpool.tile([128, 2, CW], BF16, tag=f"h{c}",
                                    name=f"h0_{c}")
                nc.vector.memset(h_t[c][:], 0.0)
                an_t[c] = hpool.tile([128, 2, CW], BF16, tag=f"an{c}",
                                     name=f"an0_{c}")
                nc.vector.memset(an_t[c][:], 0.0)

            def xstripe(gri, gin, g0, gn, stripes):
                gw = gn * W
                if g0 * W < 512:
                    xT = xTa
                    col = ds(g0 * W, gw)
                else:
                    xT = xTb
                    col = ds(g0 * W - 512, gw)
                for s in stripes:
                    if s < 4:
                        j = s
                        nc.tensor.matmul(gri[:, j, :gn, :],
                                         brow[0:1, ts(j, 128)],
                                         ones[0:1, :gw], start=True,
                                         stop=False)
                        for k in range(2):
                            nc.tensor.matmul(
                                gri[:, j, :gn, :], wT[:, k, ts(j, 128)],
                                xT[:, k, col], start=False, stop=(k == 1))
                    else:
                        j = s - 4
                        nc.tensor.matmul(gin[:, j, :gn, :],
                                         brow[0:1, ds(512 + j * 128, 128)],
                                         ones[0:1, :gw], start=True,
                                         stop=False)
                        for k in range(2):
                            nc.tensor.matmul(
                                gin[:, j, :gn, :],
                                wT[:, k, ds(512 + j * 128, 128)],
                                xT[:, k, col], start=False, stop=(k == 1))

            def newgroup():
                gri = gx.tile([128, 4, GS, W], F32, tag="gri", name="gri")
                gin = gx.tile([128, 2, GS, W], F32, tag="gin", name="gin")
                return gri, gin

            def gin_copy(gin, gn):
                gs_sb = ew.tile([128, 2, GS, W], BF16, tag="ginS", name="ginS")
                nc.vector.tensor_copy(gs_sb[:, :, :gn, :], gin[:, :, :gn, :])
                return gs_sb

            cur = newgroup()
            xstripe(*cur, GROUPS[0][0], GROUPS[0][1], range(6))
            cur_sb = gin_copy(cur[1], GROUPS[0][1])
            nxt = newgroup() if len(GROUPS) > 1 else None
            nxt_sb = None
            # stripes of the next group, spread over this group's steps
            STRIPE_SCHED = {0: (0, 1), 1: (2, 3), 2: (4,), 3: (5,)}

            for gi, (g0, gn) in enumerate(GROUPS):
                gri, gin = cur
                for tl in range(gn):
                    t = g0 + tl
                    # prefetch stripes of the next x group FIRST so they sit
                    # ahead of the gate matmuls in the PE queue and drain
                    # during this step's elementwise phase
                    if nxt is not None and gi + 1 < len(GROUPS):
                        g0n, gnn = GROUPS[gi + 1]
                        sched = STRIPE_SCHED.get(tl, ())
                        if gn < GS and tl == gn - 1:
                            sched = tuple(s for tl2 in range(tl, GS)
                                          for s in STRIPE_SCHED.get(tl2, ()))
                        xstripe(*nxt, g0n, gnn, sched)
                    for c in range(2):
                        bsl = ds(c * CW, CW)
                        ps_ghn = ghnp.tile([128, 2, CW], F32, tag=f"ghn{c}",
                                           name=f"ghn{c}")
                        # r tiles: m-src first (ready early), then an-src
                        for src in (m_t[c], an_t[c]):
                            for j in range(2):
                                for k in range(2):
                                    nc.tensor.matmul(
                                        gri[:, j, tl, bsl],
                                        wT[:, k, ds(768 + j * 128, 128)],
                                        src[:, k, :], start=False, stop=False,
                                        skip_group_check=True)
                        for src in (m_t[c], an_t[c]):
                            for j in range(2, 4):
                                for k in range(2):
                                    nc.tensor.matmul(
                                        gri[:, j, tl, bsl],
                                        wT[:, k, ds(768 + j * 128, 128)],
                                        src[:, k, :], start=False, stop=False,
                                        skip_group_check=True)
                        for j in range(2):
                            nc.tensor.matmul(
                                ps_ghn[:, j, :],
                                brow[0:1, ds(768 + j * 128, 128)],
                                ones[0:1, :CW], start=True, stop=False)
                            for si, src in enumerate((m_t[c], an_t[c])):
                                for k in range(2):
                                    nc.tensor.matmul(
                                        ps_ghn[:, j, :],
                                        wT[:, k, ds(1280 + j * 128, 128)],
                                        src[:, k, :],
                                        start=False,
                                        stop=(si == 1 and k == 1))
                        sri = ew.tile([128, 4, CW], BF16, tag=f"sri{c}",
                                      name=f"sri{c}")
                        nc.scalar.activation(sri[:], gri[:, :, tl, bsl],
                                             AF.Sigmoid)
                        u1 = ew.tile([128, 2, CW], BF16, tag=f"u1{c}",
                                     name=f"u1{c}")
                        nc.vector.tensor_tensor(
                            u1[:], ps_ghn[:], sri[:, 0:2, :], OP.mult)
                        u2 = ew.tile([128, 2, CW], BF16, tag=f"u2{c}",
                                     name=f"u2{c}")
                        nc.vector.tensor_tensor(u2[:], u1[:],
                                                cur_sb[:, :, tl, bsl], OP.add)
                        a_t = ew.tile([128, 2, CW], BF16, tag=f"a{c}",
                                      name=f"a{c}")
                        nc.vector.tensor_tensor(
                            a_t[:], sri[:, 2:4, :],
                            cf_slice(t, c).to_broadcast((128, 2, CW)),
                            OP.mult)
                        n_t = ew.tile([128, 2, CW], BF16, tag=f"n{c}",
                                      name=f"n{c}")
                        nc.scalar.activation(n_t[:], u2[:], AF.Tanh)
                        # off-critical: q, m (Pool)
                        q_t = ew.tile([128, 2, CW], BF16, tag=f"q{c}",
                                      name=f"q{c}")
                        nc.gpsimd.tensor_scalar(q_t[:], a_t[:], -1.0, 1.0,
                                                OP.mult, OP.add)
                        m_new = hpool.tile([128, 2, CW], BF16, tag=f"m{c}",
                                           name=f"m{c}")
                        nc.gpsimd.tensor_tensor(m_new[:], h_t[c][:], q_t[:],
                                                OP.mult)
                        # critical: an
                        an_new = hpool.tile([128, 2, CW], BF16, tag=f"an{c}",
                                            name=f"an{c}")
                        nc.vector.tensor_tensor(an_new[:], a_t[:], n_t[:],
                                                OP.mult)
                        # off-critical: h materialization (Pool)
                        h_new = hpool.tile([128, 2, CW], BF16, tag=f"h{c}",
                                           name=f"h{c}")
                        nc.gpsimd.tensor_tensor(h_new[:], m_new[:], an_new[:],
                                                OP.add)
                        m_t[c], an_t[c], h_t[c] = m_new, an_new, h_new
                # rotate groups
                if gi + 1 < len(GROUPS):
                    cur = nxt
                    cur_sb = gin_copy(cur[1], GROUPS[gi + 1][1])
                    nxt = newgroup() if gi + 2 < len(GROUPS) else None

            for c in range(2):
                nc.sync.dma_start(
                    hn_out.ap().rearrange("p (k b) -> p k b", k=2)
                    [:, :, ds(c * CW, CW)], h_t[c][:])

    nc.compile()
    return nc


def scan_host_inputs(seq, emb, emb_purpose, w_ih, w_hh, b_ih, b_hh):
    seq = np.asarray(seq)
    xg = emb[seq]                      # [S, B, D] gather (input staging)
    in_maps = []
    for c in range(NCORES):
        p, h = CORE_PH[c]
        sl = slice(h * W, (h + 1) * W)
        xh = xg[:, sl, :]              # [S, W, D]
        xT = np.ascontiguousarray(
            xh.transpose(2, 0, 1).reshape(2, 128, SB).transpose(1, 0, 2))
        wiT = np.ascontiguousarray(
            w_ih[p].T.reshape(2, 128, 768).transpose(1, 0, 2))
        whT = np.ascontiguousarray(
            w_hh[p].T.reshape(2, 128, 768).transpose(1, 0, 2))
        perm = [p, (p + 1) % 3, (p + 2) % 3]
        pT = np.ascontiguousarray(
            emb_purpose[perm].T.reshape(2, 128, 3).transpose(1, 0, 2))
        m = (seq[:, sl] != 0).astype(np.float32).reshape(SB)
        mask = np.ascontiguousarray(m.reshape(NT, 128).T)
        bsum = (b_ih[p] + b_hh[p])[:512]
        pm = np.concatenate(
            [pT.reshape(128, 6), mask], axis=1).astype(_BF)
        xTbf = xT.astype(_BF)
        brow = np.concatenate(
            [bsum, b_ih[p][512:], b_hh[p][512:]])[None, :].astype(_BF)
        wT = np.concatenate([wiT, whT], axis=2).astype(_BF)
        in_maps.append({
            "pm": pm, "brow": brow, "xTa": np.ascontiguousarray(xTbf[:, :, :512]),
            "xTb": np.ascontiguousarray(xTbf[:, :, 512:]), "wT": wT,
        })
    return in_maps


# --------------------------------------------------------------------------
# Launch 2: scoring
# --------------------------------------------------------------------------

def build_score_nc():
    nc = bacc.Bacc("TRN2", target_bir_lowering=False, debug=False,
                   num_devices=NCORES)

    # hT6: [128, 6, 128] = (d-part, [hn2 k0, hn2 k1, d0 k0, d0 k1, d1 k0,
    #                                d1 k1], b) where d_p = hn_p - hn_2
    hT6_d = nc.dram_tensor("hT6", [128, 6, 128], BF16, kind="ExternalInput")
    eT_d = nc.dram_tensor("eT", [128, 2, T_PAD], BF16, kind="ExternalInput")
    w01_d = nc.dram_tensor("w01", [1, 2, T_PAD], BF16, kind="ExternalInput")
    scores_d = nc.dram_tensor("scores", [128, T_PAD], F32,
                              kind="ExternalOutput")

    QBOUNDS = [0, 512, 1536, 3072, 4608, T_PAD]

    with tile.TileContext(nc) as tc:
        with (
            tc.tile_pool(name="consts", bufs=1) as consts,
            tc.tile_pool(name="wpsum", bufs=3, space="PSUM") as wpsum,
            tc.tile_pool(name="spsum", bufs=2, space="PSUM") as spsum,
            tc.tile_pool(name="work", bufs=4) as work,
        ):
            hT6 = consts.tile_from(hT6_d.ap())
            w01 = consts.tile_from(w01_d.ap())
            onek = consts.tile([1, 128], BF16)
            nc.vector.memset(onek[:], 1.0)
            # quarter the eT load so chunk 0 can start after ~1/4 of the DMA
            eT_q = []
            for q in range(5):
                lo, hi = QBOUNDS[q], QBOUNDS[q + 1]
                eq = consts.tile([128, 2, hi - lo], BF16, name=f"eq{q}")
                nc.sync.dma_start(eq[:], eT_d.ap()[:, :, ds(lo, hi - lo)])
                eT_q.append(eq)

            def esl(c0, cs):
                q = 0
                while QBOUNDS[q + 1] <= c0:
                    q += 1
                assert c0 + cs <= QBOUNDS[q + 1]
                return eT_q[q][:, :, ds(c0 - QBOUNDS[q], cs)]

            for ci, (c0, cs) in enumerate(zip(CHUNK_OFFS[:-1], CHUNK_SIZES)):
                # broadcast w0,w1 rows across partitions (K=1 ones-matmul)
                ps_w = wpsum.tile([128, 2, NC], F32, tag="w", name="ps_w")
                for p in range(2):
                    nc.tensor.matmul(ps_w[:, p, :cs], onek[0:1, :],
                                     w01[0:1, p, ds(c0, cs)],
                                     start=True, stop=True)
                w_sb = work.tile([128, 2, NC], BF16, tag="wsb", name="w_sb")
                nc.scalar.activation(w_sb[:, :, :cs], ps_w[:, :, :cs], AF.Copy)
                # scaled tables: esc[p,k,:] = eT[k,:] * w_p
                # (4 plain TTs, no broadcast APs, so DVE 2x mode applies;
                #  one TT on Pool to balance engines)
                esc = work.tile([128, 2, 2, NC], BF16, tag="esc", name="esc")
                echunk = esl(c0, cs)
                for p in range(2):
                    for k in range(2):
                        eng = nc.gpsimd if (p == 1 and k == 1) else nc.vector
                        eng.tensor_tensor(
                            esc[:, p, k, :cs], echunk[:, k, :],
                            w_sb[:, p, :cs], OP.mult)
                # scores = P2 + w0*dP0 + w1*dP1, all in one PSUM accumulation
                ps_s = spsum.tile([128, NC], F32, tag="s", name="ps_s")
                for k in range(2):
                    nc.tensor.matmul(ps_s[:, :cs], hT6[:, k, :],
                                     echunk[:, k, :],
                                     start=(k == 0), stop=False)
                for p in range(2):
                    for k in range(2):
                        nc.tensor.matmul(
                            ps_s[:, :cs], hT6[:, 2 + p * 2 + k, :],
                            esc[:, p, k, :cs],
                            start=False, stop=(p == 1 and k == 1))
                out_c = work.tile([128, NC], F32, tag="out", name="out_c")
                nc.vector.tensor_copy(out_c[:, :cs], ps_s[:, :cs])
                nc.sync.dma_start(scores_d.ap()[:, ds(c0, cs)], out_c[:, :cs])

    nc.compile()
    return nc


def score_host_inputs(hn_bf, emb, emb_purpose):
    # host: tcw softmax (z = emb @ ep.T is 0.2% of total FLOPs) + hn deltas
    z = emb @ emb_purpose.T                      # [T, 3] f32
    z = z - z.max(axis=1, keepdims=True)
    ez = np.exp(z)
    w = ez / ez.sum(axis=1, keepdims=True)       # tcw

    hn = hn_bf.astype(np.float32)                # [3, B, D]
    h2 = hn[2]
    d0 = hn[0] - h2
    d1 = hn[1] - h2
    # hT6 [128, 6, 128]: stationary tiles [d-part, slot, b]
    hT6 = np.zeros((128, 6, 128), _BF)
    for k in range(2):
        hT6[:, 0 + k, :] = h2.T[k * 128:(k + 1) * 128, :].astype(_BF)
        hT6[:, 2 + k, :] = d0.T[k * 128:(k + 1) * 128, :].astype(_BF)
        hT6[:, 4 + k, :] = d1.T[k * 128:(k + 1) * 128, :].astype(_BF)

    embT = emb.T.astype(_BF)  # [256, 50001]

    base = N_ITEMS // NCORES
    rem = N_ITEMS - base * NCORES
    bounds = []
    s0 = 0
    for c in range(NCORES):
        n = base + (1 if c < rem else 0)
        bounds.append((s0, s0 + n))
        s0 += n

    in_maps = []
    for c in range(NCORES):
        lo, hi = bounds[c]
        n = hi - lo
        eT = np.zeros((128, 2, T_PAD), _BF)
        eT[:, :, :n] = embT[:, lo:hi].reshape(2, 128, n).transpose(1, 0, 2)
        w01 = np.zeros((1, 2, T_PAD), _BF)
        w01[0, :, :n] = w[lo:hi, 0:2].T.astype(_BF)
        in_maps.append({"hT6": hT6, "eT": eT, "w01": w01})
    return in_maps, bounds


# --------------------------------------------------------------------------
# Entry point
# --------------------------------------------------------------------------

_SCAN_NC = None
_SCORE_NC = None


def _get_ncs():
    global _SCAN_NC, _SCORE_NC
    if _SCAN_NC is None:
        _SCAN_NC = build_scan_nc()
    if _SCORE_NC is None:
        _SCORE_NC = build_score_nc()
    return _SCAN_NC, _SCORE_NC


def kernel(seq, emb, emb_purpose, w_ih, w_hh, b_ih, b_hh):
    seq = np.asarray(seq)
    emb = np.asarray(emb, np.float32)
    emb_purpose = np.asarray(emb_purpose, np.float32)
    w_ih = np.asarray(w_ih, np.float32)
    w_hh = np.asarray(w_hh, np.float32)
    b_ih = np.asarray(b_ih, np.float32)
    b_hh = np.asarray(b_hh, np.float32)

    scan_nc, score_nc = _get_ncs()

    scan_ins = scan_host_inputs(seq, emb, emb_purpose, w_ih, w_hh, b_ih, b_hh)
    res1 = run_bass_kernel_spmd(scan_nc, scan_ins, core_ids=list(range(NCORES)))

    hn = np.zeros((3, B, DIM), _BF)
    for c in range(6):
        p, h = CORE_PH[c]
        sl = res1.results[c]["hn_out"].reshape(128, 2, W)
        for k in range(2):
            hn[p, h * W:(h + 1) * W, k * 128:(k + 1) * 128] = sl[:, k, :].T

    score_ins, bounds = score_host_inputs(hn, emb, emb_purpose)
    res2 = run_bass_kernel_spmd(score_nc, score_ins,
                                core_ids=list(range(NCORES)))

    scores = np.empty((B, N_ITEMS), np.float32)
    for c in range(NCORES):
        lo, hi = bounds[c]
        scores[:, lo:hi] = res2.results[c]["scores"][:, : hi - lo]
    return scores
